# revision 34
# baseline (speedup 1.0000x reference)
"""Trainium2 Bass kernel for nn_GAT_NFM (2x GAT encoder layers + NFM bilinear
pooling + projection) on 8 NeuronCores.

Sharding: nodes are partitioned contiguously across the 8 cores (N/8 each);
edges are partitioned by src node (the segment/aggregation axis). Each core
computes its shard of the per-layer transformed features; the shards are
all-gathered into a full per-core bf16 feature table in HBM (pair-shared);
each core then gathers its edges' dst rows from that table (dma_gather) and
scatter-adds w-weighted dst rows per src node with a w-weighted one-hot
matmul on the TensorEngine: out[i] = (sum_e w_e*Hw[dst_e]) / (sum_e w_e),
where w = exp(sigmoid(edge_val * (f1[src] + f2[dst]))) (the segment-max in
the reference softmax cancels algebraically).

Layer 1 specifics: f1/f2 depend only on the input x, so the per-edge weights
w0 AND the per-src softmax denominators are precomputed on the host and
streamed as dense per-slot metadata ([128, TOT] columns); the layer-1 feature
table row is exactly Hw0 (256 bf16 = 512B, the dma_gather elem quantum).
The layer-1 output is normalized, transposed and immediately pushed through
W1ext = [W1 | W1@v11 | W1@v10] to build the layer-2 table (no HBM roundtrip
of H1).  Layer 2: table row = [Hw1 (128) | f2' | 1 | pad] (512B; the ones
column folds the denominator into the aggregation matmul), f1' comes from a
core-local 256B-row scalar table gathered by local src index, and
exp(sigmoid(x)) is computed as exp(0.5*tanh(0.5x)+0.5) so both activations
live in one ACT table set.  Edges are grouped (core, src-tile, dst-bucket)
— buckets keep dma_gather indices within int16 — sorted by dst inside each
group for HBM page locality, padded to multiples of 128 with idx=-1 slots
(trailing negative indices are skipped by the gather HW).  NFM is computed
in the dense phase and kept SBUF-resident until the fused projection.
"""

import math
import os

import numpy as np

import concourse.bass as bass
import concourse.bacc as bacc
import concourse.mybir as mybir
import concourse.tile as tile
from concourse.bass_utils import run_bass_kernel_spmd
from concourse.masks import make_identity

P = 128
N_CORES = 8
F32 = mybir.dt.float32
BF16 = mybir.dt.bfloat16
I32 = mybir.dt.int32
I16 = mybir.dt.int16
AF = mybir.ActivationFunctionType
OP = mybir.AluOpType


# ----------------------------------------------------------------- host prep

def _prep(inputs, n_cores=N_CORES, bucket_cap=25000):
    x = np.ascontiguousarray(np.asarray(inputs["x"], dtype=np.float32))
    ev = np.asarray(inputs["edge_val"], dtype=np.float32)
    src = np.asarray(inputs["edge_src"], dtype=np.int64)
    dst = np.asarray(inputs["edge_dst"], dtype=np.int64)
    W0 = np.asarray(inputs["W0"], dtype=np.float32)
    W1 = np.asarray(inputs["W1"], dtype=np.float32)
    v00 = np.asarray(inputs["v0_0"], dtype=np.float32)
    v01 = np.asarray(inputs["v0_1"], dtype=np.float32)
    v10 = np.asarray(inputs["v1_0"], dtype=np.float32)
    v11 = np.asarray(inputs["v1_1"], dtype=np.float32)
    fme = np.asarray(inputs["fm_emb"], dtype=np.float32)
    pjw = np.asarray(inputs["proj_W"], dtype=np.float32)
    pjb = np.asarray(inputs["proj_b"], dtype=np.float32)

    N, Din = x.shape
    E = src.shape[0]
    D0 = W0.shape[1]          # 256
    D1 = W1.shape[1]          # 128
    FM = fme.shape[1]         # 64
    NCLS = pjw.shape[1]       # 64
    assert N % n_cores == 0
    NSH = N // n_cores
    NT = math.ceil(NSH / P)
    assert NSH < (1 << 15), "local scalar-gather index must fit int16"

    # layer-1 attention is a pure function of x: precompute per-edge weights
    # w0 = exp(sigmoid(ev*(f1[src]+f2[dst]))) and per-src denominators.
    f1 = x @ (W0 @ v00)[:, 0]
    f2 = x @ (W0 @ v01)[:, 0]
    lg = ev * (f1[src] + f2[dst])
    w0 = np.exp(1.0 / (1.0 + np.exp(-lg))).astype(np.float32)
    den0 = np.bincount(src, weights=w0.astype(np.float64), minlength=N)
    rec0_n = (1.0 / np.maximum(den0, 1e-30)).astype(np.float32)

    # feature-table row widths (bf16 elements; 512B gather quantum)
    C0 = D0                   # 256: [Hw0]
    C1 = 2 * D1               # 256: [Hw1 | f2' | 1 | pad]
    CS = 128                  # local scalar-table row (256B)

    # dst buckets (int16 index range)
    NB = max(1, math.ceil(N / min(bucket_cap, 32000)))
    BSZ = math.ceil(N / NB)

    # ---- edge grouping: (core, src-tile, dst-bucket), dst-sorted in group
    loc = src % NSH
    core_of = src // NSH
    ltile = loc // P
    buck = dst // BSZ
    gid = (core_of * NT + ltile) * NB + buck
    order = np.argsort(gid * np.int64(N) + dst, kind="stable")
    sdst = dst[order]
    sgid = gid[order]
    sloc = loc[order]
    sev = ev[order]
    sw0 = w0[order]

    cnt = np.bincount(sgid, minlength=n_cores * NT * NB)
    cntc = cnt.reshape(n_cores, NT, NB)
    SZ = np.maximum(P, ((cntc.max(axis=0) + P - 1) // P) * P)  # [NT, NB]
    TPT = (SZ.sum(axis=1) // P).astype(np.int64)               # [NT]
    CUM = np.zeros(NT + 1, np.int64)
    CUM[1:] = np.cumsum(TPT)
    TOT = int(CUM[-1])                                         # cols per core
    TOTS = TOT * P                                             # slots per core
    OFF = np.zeros((NT, NB), np.int64)
    run = 0
    for nt in range(NT):
        for b in range(NB):
            OFF[nt, b] = run
            run += SZ[nt, b]
    assert run == TOTS

    grp = np.zeros(n_cores * NT * NB + 1, np.int64)
    grp[1:] = np.cumsum(cntc.reshape(-1))
    within = np.arange(E, dtype=np.int64) - grp[sgid]
    snt = (sgid // NB) % NT
    sb = sgid % NB
    pad_pos = OFF[snt, sb] + within

    # pad slots gather row 0 of their bucket (valid index; the one-hot masks
    # them via srel=-1/w=0). num_idxs per (tile, bucket) call is trimmed to
    # the max real count over cores, 16-aligned (MX16), so the 128-rounding
    # tail of each group is never gathered.
    MX16 = ((cntc.max(axis=0) + 15) // 16) * 16                # [NT, NB]
    dst16 = np.zeros((n_cores, TOTS), np.int16)
    src16 = np.zeros((n_cores, TOTS), np.int16)
    srel = np.full((n_cores, TOTS), -1.0, np.float32)
    w0s = np.zeros((n_cores, TOTS), np.float32)
    aval = np.zeros((n_cores, TOTS), np.float32)
    ci = core_of[order]
    dst16[ci, pad_pos] = (sdst - sb * BSZ).astype(np.int16)
    src16[ci, pad_pos] = sloc.astype(np.int16)
    srel[ci, pad_pos] = (sloc % P).astype(np.float32)
    w0s[ci, pad_pos] = sw0
    aval[ci, pad_pos] = sev

    def to_cols(a, dt):    # [TOTS] slot-major -> [P, TOT] (slot = col*128+p)
        return np.ascontiguousarray(a.reshape(TOT, P).T.astype(dt))

    def to_wrap16(a):      # [TOTS] -> [128, TOTS//16] 16-wrapped + replicated
        w = np.ascontiguousarray(a.reshape(TOTS // 16, 16).T)
        return np.ascontiguousarray(np.tile(w, (8, 1)))

    # tiny replicated weights
    w0e = np.ascontiguousarray(W0)                                # [Din, D0]
    w1e = np.ascontiguousarray(
        np.concatenate([W1, W1 @ v11, W1 @ v10], axis=1))         # [D0, D1+2]
    ee2 = np.ascontiguousarray(
        np.concatenate([fme, fme * fme], axis=1))                 # [Din, 2FM]
    pja = np.ascontiguousarray(pjw[:D1])                          # [D1, NCLS]
    pjbm = np.ascontiguousarray(0.5 * pjw[D1:])                   # [FM, NCLS]
    pbias = np.ascontiguousarray(pjb[None, :])                    # [1, NCLS]
    iota = np.broadcast_to(np.arange(P, dtype=np.float32), (P, P)).copy()

    import ml_dtypes

    def cast_bf16(a):
        return np.ascontiguousarray(a.astype(ml_dtypes.bfloat16))

    in_maps = []
    for c in range(n_cores):
        xt = np.ascontiguousarray(x[c * NSH:(c + 1) * NSH].T)     # [Din, NSH]
        rec0 = np.ones((P, NT), np.float32)
        rsh = rec0_n[c * NSH:(c + 1) * NSH]
        full = (NSH // P) * P
        rec0[:, :NSH // P] = rsh[:full].reshape(NSH // P, P).T
        if NSH % P:
            rec0[:NSH % P, NSH // P] = rsh[full:]
        in_maps.append({
            "xt": cast_bf16(xt),
            "idxf": to_wrap16(dst16[c]),
            "idxs": to_wrap16(src16[c]),
            "srel": to_cols(srel[c], np.float32),
            "w0s": to_cols(w0s[c], np.float32),
            "aval": cast_bf16(to_cols(aval[c], np.float32)),
            "rec0": rec0,
            "w0e": cast_bf16(w0e), "w1e": w1e, "ee2": cast_bf16(ee2),
            "pja": pja, "pjb": pjbm, "pbias": pbias,
            "iota": iota,
        })

    cfg = dict(N=N, E=E, Din=Din, D0=D0, D1=D1, FM=FM, NCLS=NCLS,
               NSH=NSH, NT=NT, NB=NB, BSZ=BSZ,
               SZ=[[int(v) for v in row] for row in SZ],
               OFF=[[int(v) for v in row] for row in OFF],
               TPT=[int(t) for t in TPT], CUM=[int(c) for c in CUM],
               TOT=TOT, C0=C0, C1=C1, CS=CS, n_cores=n_cores,
               MX=[[int(v) for v in row] for row in MX16])
    return cfg, in_maps


# ------------------------------------------------------------ device program

def _build(cfg, reps=1):
    N = cfg["N"]; Din = cfg["Din"]; D0 = cfg["D0"]; D1 = cfg["D1"]
    FM = cfg["FM"]; NCLS = cfg["NCLS"]; NSH = cfg["NSH"]; NT = cfg["NT"]
    NB = cfg["NB"]; BSZ = cfg["BSZ"]; SZ = cfg["SZ"]; OFF = cfg["OFF"]
    TPT = cfg["TPT"]; CUM = cfg["CUM"]; TOT = cfg["TOT"]; MX = cfg["MX"]
    C0 = cfg["C0"]; C1 = cfg["C1"]; CS = cfg["CS"]; n_cores = cfg["n_cores"]
    TPTmax = max(TPT)
    KD = Din // P             # 4
    KD0 = D0 // P             # 2
    FM2 = 2 * FM

    CH = int(os.environ.get("KCHUNK", "4096"))
    kfp8 = int(os.environ.get("KFP8", "0"))   # 0=bf16, 1=L2 fp8, 2=both fp8
    FP8 = mybir.dt.float8e4
    DT0 = FP8 if kfp8 >= 2 else BF16          # layer-1 table dtype
    DT1 = FP8 if kfp8 >= 1 else BF16          # layer-2 table dtype
    nqueues = int(os.environ.get("KQUEUES", "1"))
    # NOTE: neuronxcc's BIR verifier requires CollectiveCompute outputs to be
    # contiguous, so the all-gathers cannot be row-split (strided outputs).
    shared = os.environ.get("KSHARED", "0") == "1"
    agsplit = not shared and os.environ.get("KAGSPLIT", "0") == "1"
    nc = bacc.Bacc("TRN2", target_bir_lowering=False, debug=False,
                   num_devices=n_cores, num_swdge_queues=nqueues)
    qrr = [0]

    xt_d = nc.dram_tensor("xt", [Din, NSH], BF16, kind="ExternalInput")
    idxf_d = nc.dram_tensor("idxf", [P, TOT * 8], I16, kind="ExternalInput")
    idxs_d = nc.dram_tensor("idxs", [P, TOT * 8], I16, kind="ExternalInput")
    srel_d = nc.dram_tensor("srel", [P, TOT], F32, kind="ExternalInput")
    w0s_d = nc.dram_tensor("w0s", [P, TOT], F32, kind="ExternalInput")
    aval_d = nc.dram_tensor("aval", [P, TOT], BF16, kind="ExternalInput")
    rec0_d = nc.dram_tensor("rec0", [P, NT], F32, kind="ExternalInput")
    w0e_d = nc.dram_tensor("w0e", [Din, D0], BF16, kind="ExternalInput")
    w1e_d = nc.dram_tensor("w1e", [D0, D1 + 2], F32, kind="ExternalInput")
    ee2_d = nc.dram_tensor("ee2", [Din, FM2], BF16, kind="ExternalInput")
    pja_d = nc.dram_tensor("pja", [D1, NCLS], F32, kind="ExternalInput")
    pjb_d = nc.dram_tensor("pjb", [FM, NCLS], F32, kind="ExternalInput")
    pbias_d = nc.dram_tensor("pbias", [1, NCLS], F32, kind="ExternalInput")
    iota_d = nc.dram_tensor("iota", [P, P], F32, kind="ExternalInput")
    out_d = nc.dram_tensor("out", [NSH, NCLS], F32, kind="ExternalOutput")

    def tw(nt):
        return min(P, NSH - nt * P)

    HALF = (NSH // 2 // P) * P            # AG split row boundary (tile-align)
    half_tile = HALF // P - 1             # last tile fully inside first half

    with tile.TileContext(nc) as tc:
        with tc.tile_pool(name="dram", bufs=1, space="DRAM") as dram, \
             tc.tile_pool(name="const", bufs=1) as cpool, \
             tc.tile_pool(name="meta", bufs=1) as mpool:

            aspace = "Shared" if shared else "Local"
            T0L = dram.tile([NSH, C0], DT0)
            T0F = dram.tile([n_cores, NSH, C0], DT0, addr_space=aspace)
            T1L = dram.tile([NSH, C1], DT1)
            T1F = dram.tile([n_cores, NSH, C1], DT1, addr_space=aspace)
            T1S = dram.tile([NSH, CS], BF16)
            NFMT = dram.tile([FM, NSH], F32)

            # constants
            iota_t = cpool.tile([P, P], F32)
            nc.sync.dma_start(out=iota_t[:], in_=iota_d[:, :])
            ident = cpool.tile([P, P], F32)
            make_identity(nc, ident[:])
            ones_row = cpool.tile([1, P], F32)
            nc.vector.memset(ones_row[:], 1.0)
            half_col = cpool.tile([P, 1], F32)
            nc.vector.memset(half_col[:], 0.5)
            w0e_t = [cpool.tile([P, D0], BF16, tag=f"w0e{k}", name=f"w0e{k}")
                     for k in range(KD)]
            for k in range(KD):
                nc.sync.dma_start(out=w0e_t[k][:], in_=w0e_d[k * P:(k + 1) * P, :])
            w1e_t = [cpool.tile([P, D1 + 2], F32, tag=f"w1e{k}", name=f"w1e{k}")
                     for k in range(KD0)]
            for k in range(KD0):
                nc.sync.dma_start(out=w1e_t[k][:], in_=w1e_d[k * P:(k + 1) * P, :])
            ee2_t = [cpool.tile([P, FM2], BF16, tag=f"ee2{k}", name=f"ee2{k}")
                     for k in range(KD)]
            for k in range(KD):
                nc.sync.dma_start(out=ee2_t[k][:], in_=ee2_d[k * P:(k + 1) * P, :])
            pja_t = cpool.tile([D1, NCLS], F32)
            nc.sync.dma_start(out=pja_t[:], in_=pja_d[:, :])
            pjb_t = cpool.tile([FM, NCLS], F32)
            nc.sync.dma_start(out=pjb_t[:], in_=pjb_d[:, :])
            pbias_t = cpool.tile([1, NCLS], F32)
            nc.sync.dma_start(out=pbias_t[:], in_=pbias_d[:, :])

            # per-slot metadata, resident for the whole run
            srel_t = mpool.tile([P, TOT], F32)
            w0s_t = mpool.tile([P, TOT], F32)
            aval_t = mpool.tile([P, TOT], BF16)
            rec0_t = mpool.tile([P, NT], F32)
            nc.sync.dma_start(out=srel_t[:], in_=srel_d[:, :])
            nc.sync.dma_start(out=w0s_t[:], in_=w0s_d[:, :])
            nc.sync.dma_start(out=aval_t[:], in_=aval_d[:, :])
            nc.sync.dma_start(out=rec0_t[:], in_=rec0_d[:, :])
            # layer-2 f1'[src] per slot, filled by the scalar-gather prepass
            # that overlaps the T1 all-gather
            f1all = mpool.tile([P, TOT], F32)

            def ag(inp, outp, label):
                nc.gpsimd.collective_compute(
                    "AllGather", OP.bypass,
                    replica_groups=[list(range(n_cores))],
                    ins=[inp.opt()], outs=[outp.opt()])

            def _body():
                # -------- phase A: T0 rows = x @ W0 (bf16); NFM into SBUF
                with tc.tile_pool(name="a_sb", bufs=3) as asb, \
                     tc.tile_pool(name="a_xt", bufs=2) as axt, \
                     tc.tile_pool(name="a_ps", bufs=2, space="PSUM") as aps, \
                     tc.tile_pool(name="a_nf", bufs=2, space="PSUM") as anf:
                    for jc in range(0, NT, 4):
                        tiles = list(range(jc, min(jc + 4, NT)))
                        n0 = jc * P
                        cw = sum(tw(t) for t in tiles)
                        xtm = axt.tile([P, KD, 4 * P], BF16, tag="xt")
                        nc.sync.dma_start(
                            out=xtm[:, :, :cw],
                            in_=xt_d[:, n0:n0 + cw].rearrange(
                                "(k p) c -> p k c", p=P))
                        off = 0
                        for t in tiles:
                            wm = tw(t)
                            ap_ = aps.tile([P, D0], F32, tag="aps", space="PSUM")
                            for k in range(KD):
                                nc.tensor.matmul(out=ap_[:wm, :],
                                                 lhsT=xtm[:, k, off:off + wm],
                                                 rhs=w0e_t[k][:],
                                                 start=(k == 0), stop=(k == KD - 1))
                            st = asb.tile([P, C0], DT0, tag="st")
                            nc.vector.tensor_copy(out=st[:wm, :], in_=ap_[:wm, :])
                            nc.sync.dma_start(out=T0L[t * P:t * P + wm, :],
                                              in_=st[:wm, :])
                            off += wm
                        if agsplit and tiles[0] <= half_tile < tiles[-1] + 1:
                            ag(T0L[0:HALF, :], T0F[:, 0:HALF, :], "t0a")
                    if agsplit:
                        ag(T0L[HALF:NSH, :], T0F[:, HALF:NSH, :], "t0b")
                    else:
                        ag(T0L[:, :], T0F[:, :, :], "t0")
                    # NFM (independent of the tables) computed while the T0
                    # all-gather runs on the collective cores.
                    for jc in range(0, NT, 4):
                        tiles = list(range(jc, min(jc + 4, NT)))
                        n0 = jc * P
                        cw = sum(tw(t) for t in tiles)
                        xtm = axt.tile([P, KD, 4 * P], BF16, tag="xt")
                        nc.sync.dma_start(
                            out=xtm[:, :, :cw],
                            in_=xt_d[:, n0:n0 + cw].rearrange(
                                "(k p) c -> p k c", p=P))
                        nf1 = anf.tile([FM, 4 * P], F32, tag="nf1", space="PSUM")
                        nf2 = anf.tile([FM, 4 * P], F32, tag="nf2", space="PSUM")
                        for k in range(KD):
                            nc.tensor.matmul(out=nf1[:, :cw], lhsT=ee2_t[k][:, :FM],
                                             rhs=xtm[:, k, :cw],
                                             start=(k == 0), stop=(k == KD - 1))
                        for k in range(KD):
                            nc.tensor.matmul(out=nf2[:, :cw],
                                             lhsT=ee2_t[k][:, FM:FM2],
                                             rhs=xtm[:, k, :cw],
                                             start=(k == 0), stop=(k == KD - 1))
                        s1 = asb.tile([FM, 4 * P], F32, tag="nfs1")
                        nc.vector.tensor_copy(out=s1[:, :cw], in_=nf1[:, :cw])
                        nfo = asb.tile([FM, 4 * P], F32, tag="nfo")
                        nc.vector.tensor_tensor(out=nfo[:, :cw], in0=s1[:, :cw],
                                                in1=s1[:, :cw], op=OP.mult)
                        nfm = asb.tile([FM, 4 * P], F32, tag="nfm")
                        nc.vector.tensor_tensor(out=nfm[:, :cw],
                                                in0=nfo[:, :cw],
                                                in1=nf2[:, :cw], op=OP.subtract)
                        nc.sync.dma_start(out=NFMT[:, n0:n0 + cw],
                                          in_=nfm[:, :cw])

                # per-bucket rank-slab views: bucket b = ranks [b*rpb, (b+1)*rpb)
                assert BSZ % NSH == 0
                rpb = BSZ // NSH
                T0Fb = [T0F[b * rpb:(b + 1) * rpb, :, :]
                        .rearrange("r n c -> (r n) c") for b in range(NB)]
                T1Fb = [T1F[b * rpb:(b + 1) * rpb, :, :]
                        .rearrange("r n c -> (r n) c") for b in range(NB)]

                # -------- L1 edge phase + fused T1 build
                with tc.tile_pool(name="e_g", bufs=3) as gp, \
                     tc.tile_pool(name="e_ix", bufs=3) as ixp, \
                     tc.tile_pool(name="e_oh", bufs=4) as ohp, \
                     tc.tile_pool(name="e_ps", bufs=2, space="PSUM") as psp, \
                     tc.tile_pool(name="e_tp", bufs=2, space="PSUM") as tpp, \
                     tc.tile_pool(name="e_bp", bufs=2, space="PSUM") as bpp, \
                     tc.tile_pool(name="e_sb", bufs=3) as esb:
                    # zero the gather buffers once: pad slots are skipped by
                    # the gather (idx=-1) and must never hold non-finite bits
                    # (0 * NaN = NaN in the aggregation matmul).
                    for zi in range(3):
                        z = gp.tile([P, TPTmax, C0], DT0, tag="g",
                                    name=f"zg1_{zi}")
                        nc.vector.memset(z[:], 0.0)
                    for nt in range(NT):
                        tpt = TPT[nt]
                        c0 = CUM[nt]
                        wm = tw(nt)
                        g = gp.tile([P, TPTmax, C0], DT0, tag="g")
                        ixf = ixp.tile([P, TPTmax * 8], I16, tag="ixf")
                        nc.sync.dma_start(
                            out=ixf[:, :tpt * 8],
                            in_=idxf_d[:, c0 * 8:(c0 + tpt) * 8])
                        for b in range(NB):
                            for z0 in range(0, MX[nt][b], CH):
                                sz = min(CH, MX[nt][b] - z0)
                                o8 = (OFF[nt][b] - CUM[nt] * P + z0) // 16
                                ot = (OFF[nt][b] - CUM[nt] * P + z0) // P
                                nc.gpsimd.dma_gather(
                                    out_ap=g[:, ot:ot + (sz + P - 1) // P, :],
                                    in_ap=T0Fb[b],
                                    idxs_ap=ixf[:, o8:o8 + (sz + 15) // 16],
                                    num_idxs=sz, num_idxs_reg=sz,
                                    elem_size=C0, elem_step=C0,
                                    single_packet=False,
                                    queue_num=qrr[0] % nqueues)
                                qrr[0] += 1
                        # aggregate with w-weighted one-hots
                        ps = psp.tile([P, D0], F32, tag="ps", space="PSUM")
                        for t in range(tpt):
                            oh = ohp.tile([P, P], DT0, tag="oh")
                            nc.vector.tensor_scalar(
                                out=oh[:], in0=iota_t[:],
                                scalar1=srel_t[:, c0 + t:c0 + t + 1],
                                scalar2=w0s_t[:, c0 + t:c0 + t + 1],
                                op0=OP.is_equal, op1=OP.mult)
                            nc.tensor.matmul(out=ps[:], lhsT=oh[:],
                                             rhs=g[:, t, :],
                                             start=(t == 0), stop=(t == tpt - 1))
                        hsb = esb.tile([P, D0], F32, tag="hsb")
                        nc.vector.tensor_scalar(
                            out=hsb[:], in0=ps[:],
                            scalar1=rec0_t[:, nt:nt + 1], scalar2=None,
                            op0=OP.mult)
                        # fused: T1 row = [H1 @ W1 | f2' | 1 | pad], T1S = f1'
                        bp = bpp.tile([P, D1 + 2], F32, tag="bp", space="PSUM")
                        for k in range(KD0):
                            tp = tpp.tile([P, P], F32, tag="tp", space="PSUM")
                            nc.tensor.transpose(out=tp[:, :wm],
                                                in_=hsb[:wm, k * P:(k + 1) * P],
                                                identity=ident[:wm, :wm])
                            ht = esb.tile([P, P], F32, tag="ht")
                            nc.vector.tensor_copy(out=ht[:, :wm], in_=tp[:, :wm])
                            nc.tensor.matmul(out=bp[:wm, :], lhsT=ht[:, :wm],
                                             rhs=w1e_t[k][:],
                                             start=(k == 0), stop=(k == KD0 - 1))
                        st = esb.tile([P, C1], DT1, tag="st2")
                        nc.vector.tensor_copy(out=st[:wm, 0:D1 + 1],
                                              in_=bp[:wm, 0:D1 + 1])
                        nc.vector.memset(st[:, D1 + 1:D1 + 2], 1.0)
                        nc.vector.memset(st[:, D1 + 2:C1], 0.0)
                        nc.sync.dma_start(out=T1L[nt * P:nt * P + wm, :],
                                          in_=st[:wm, :])
                        sc = esb.tile([P, CS], BF16, tag="sc2")
                        nc.vector.memset(sc[:, :], 0.0)
                        nc.vector.tensor_copy(out=sc[:wm, 0:1],
                                              in_=bp[:wm, D1 + 1:D1 + 2])
                        nc.sync.dma_start(out=T1S[nt * P:nt * P + wm, :],
                                          in_=sc[:wm, :])
                        if agsplit and nt == half_tile:
                            ag(T1L[0:HALF, :], T1F[:, 0:HALF, :], "t1a")
                    if agsplit:
                        ag(T1L[HALF:NSH, :], T1F[:, HALF:NSH, :], "t1b")
                    else:
                        ag(T1L[:, :], T1F[:, :, :], "t1")

                # -------- L2 edge phase + fused projection
                with tc.tile_pool(name="f_g", bufs=3) as gp2, \
                     tc.tile_pool(name="p_gs", bufs=2) as gsp, \
                     tc.tile_pool(name="p_ix", bufs=2) as ixps, \
                     tc.tile_pool(name="f_ix", bufs=3) as ixp2, \
                     tc.tile_pool(name="f_w", bufs=2) as wp, \
                     tc.tile_pool(name="f_oh", bufs=4) as ohp2, \
                     tc.tile_pool(name="f_ps", bufs=2, space="PSUM") as psp2, \
                     tc.tile_pool(name="f_tp", bufs=2, space="PSUM") as tpp2, \
                     tc.tile_pool(name="f_fp", bufs=2, space="PSUM") as cfp, \
                     tc.tile_pool(name="f_sb", bufs=3) as esb2:

                    def gs_pre(nt):
                        # f1'[src] from the core-local scalar table into
                        # f1all. Depends only on T1S (ready mid-L1), so the
                        # leading K0 tiles overlap the T1 all-gather.
                        tpt = TPT[nt]
                        c0 = CUM[nt]
                        gs = gsp.tile([P, TPTmax, CS], BF16, tag="gs",
                                      name="gs")
                        ixs = ixps.tile([P, TPTmax * 8], I16, tag="ixs",
                                        name="ixs")
                        nc.sync.dma_start(
                            out=ixs[:, :tpt * 8],
                            in_=idxs_d[:, c0 * 8:(c0 + tpt) * 8])
                        for q0 in range(0, tpt, 32):
                            qn = min(32, tpt - q0)
                            nc.gpsimd.dma_gather(
                                out_ap=gs[:, q0:q0 + qn, :], in_ap=T1S[:, :],
                                idxs_ap=ixs[:, q0 * 8:(q0 + qn) * 8],
                                num_idxs=qn * P, num_idxs_reg=qn * P,
                                elem_size=CS, single_packet=False,
                                queue_num=qrr[0] % nqueues)
                            qrr[0] += 1
                        nc.vector.tensor_copy(out=f1all[:, c0:c0 + tpt],
                                              in_=gs[:, :tpt, 0])

                    # one-time zero of the gather pools so pad slots can never
                    # hold non-finite garbage (reused buffers stay finite).
                    for zi in range(3):
                        z = gp2.tile([P, TPTmax, C1], DT1, tag="g2",
                                     name=f"zg2_{zi}")
                        nc.vector.memset(z[:], 0.0)
                    for zi in range(2):
                        z2 = gsp.tile([P, TPTmax, CS], BF16, tag="gs",
                                      name=f"zgs_{zi}")
                        nc.vector.memset(z2[:], 0.0)
                    K0 = min(NT, int(os.environ.get("KPRE", "56")))
                    for nt in range(K0):
                        gs_pre(nt)
                    for nt in range(NT):
                        if K0 + nt < NT:
                            gs_pre(K0 + nt)
                        tpt = TPT[nt]
                        c0 = CUM[nt]
                        wm = tw(nt)
                        g = gp2.tile([P, TPTmax, C1], DT1, tag="g2")
                        ixf = ixp2.tile([P, TPTmax * 8], I16, tag="ixf2")
                        nc.sync.dma_start(
                            out=ixf[:, :tpt * 8],
                            in_=idxf_d[:, c0 * 8:(c0 + tpt) * 8])
                        for b in range(NB):
                            for z0 in range(0, MX[nt][b], CH):
                                sz = min(CH, MX[nt][b] - z0)
                                o8 = (OFF[nt][b] - CUM[nt] * P + z0) // 16
                                ot = (OFF[nt][b] - CUM[nt] * P + z0) // P
                                nc.gpsimd.dma_gather(
                                    out_ap=g[:, ot:ot + (sz + P - 1) // P, :],
                                    in_ap=T1Fb[b],
                                    idxs_ap=ixf[:, o8:o8 + (sz + 15) // 16],
                                    num_idxs=sz, num_idxs_reg=sz,
                                    elem_size=C1, elem_step=C1,
                                    single_packet=False,
                                    queue_num=qrr[0] % nqueues)
                                qrr[0] += 1
                        # w = exp(sigmoid(aval*(f1+f2))) = exp(.5*tanh(.5x)+.5)
                        w = wp.tile([P, TPTmax], F32, tag="w")
                        nc.vector.tensor_tensor(out=w[:, :tpt],
                                                in0=f1all[:, c0:c0 + tpt],
                                                in1=g[:, :tpt, D1], op=OP.add)
                        nc.vector.tensor_tensor(out=w[:, :tpt], in0=w[:, :tpt],
                                                in1=aval_t[:, c0:c0 + tpt],
                                                op=OP.mult)
                        nc.scalar.activation(w[:, :tpt], w[:, :tpt], AF.Tanh,
                                             scale=0.5)
                        nc.scalar.activation(w[:, :tpt], w[:, :tpt], AF.Exp,
                                             scale=0.5, bias=half_col[:, :1])
                        ps = psp2.tile([P, D1 + 2], F32, tag="ps2", space="PSUM")
                        for t in range(tpt):
                            oh = ohp2.tile([P, P], DT1, tag="oh2")
                            nc.vector.tensor_scalar(
                                out=oh[:], in0=iota_t[:],
                                scalar1=srel_t[:, c0 + t:c0 + t + 1],
                                scalar2=w[:, t:t + 1],
                                op0=OP.is_equal, op1=OP.mult)
                            nc.tensor.matmul(out=ps[:], lhsT=oh[:],
                                             rhs=g[:, t, 0:D1 + 2],
                                             start=(t == 0), stop=(t == tpt - 1))
                        den = esb2.tile([P, 1], F32, tag="den")
                        nc.vector.tensor_scalar(out=den[:], in0=ps[:, D1 + 1:D1 + 2],
                                                scalar1=1e-30, scalar2=None,
                                                op0=OP.add)
                        rec = esb2.tile([P, 1], F32, tag="rec")
                        nc.vector.reciprocal(rec[:], den[:])
                        hsb = esb2.tile([P, D1], F32, tag="hsb2")
                        nc.vector.tensor_scalar(out=hsb[:], in0=ps[:, 0:D1],
                                                scalar1=rec[:, :1], scalar2=None,
                                                op0=OP.mult)
                        # fused projection: out = [H2 | nfm] @ proj + b
                        n0 = nt * P
                        tp = tpp2.tile([P, P], F32, tag="tp2", space="PSUM")
                        nc.tensor.transpose(out=tp[:, :wm], in_=hsb[:wm, 0:D1],
                                            identity=ident[:wm, :wm])
                        h2t = esb2.tile([P, P], F32, tag="h2t")
                        nc.vector.tensor_copy(out=h2t[:, :wm], in_=tp[:, :wm])
                        nft = esb2.tile([FM, P], F32, tag="nft")
                        nc.sync.dma_start(out=nft[:, :wm], in_=NFMT[:, n0:n0 + wm])
                        fps = cfp.tile([P, NCLS], F32, tag="fps", space="PSUM")
                        nc.tensor.matmul(out=fps[:wm, :], lhsT=h2t[:, :wm],
                                         rhs=pja_t[:], start=True, stop=False)
                        nc.tensor.matmul(out=fps[:wm, :],
                                         lhsT=nft[:, :wm],
                                         rhs=pjb_t[:], start=False, stop=False)
                        nc.tensor.matmul(out=fps[:wm, :], lhsT=ones_row[:1, :wm],
                                         rhs=pbias_t[:], start=False, stop=True)
                        ot2 = esb2.tile([P, NCLS], F32, tag="ot")
                        nc.vector.tensor_copy(out=ot2[:wm, :], in_=fps[:wm, :])
                        nc.sync.dma_start(out=out_d[n0:n0 + wm, :], in_=ot2[:wm, :])

            for _rep in range(reps):
                _body()

    nc.finalize()
    return nc


_CACHE = {}


def _get_program(cfg_key, cfg):
    if cfg_key not in _CACHE:
        _CACHE[cfg_key] = _build(cfg)
    return _CACHE[cfg_key]


def kernel(**inputs) -> np.ndarray:
    cfg, in_maps = _prep(inputs)
    cfg_key = (cfg["N"], cfg["E"], cfg["Din"], cfg["D0"], cfg["D1"],
               cfg["FM"], cfg["NCLS"], tuple(cfg["TPT"]),
               tuple(tuple(r) for r in cfg["SZ"]))
    nc = _get_program(cfg_key, cfg)
    res = run_bass_kernel_spmd(nc, in_maps, list(range(cfg["n_cores"])))
    out = np.concatenate(
        [res.results[c]["out"] for c in range(cfg["n_cores"])], axis=0)
    return out.astype(np.float32)


# revision 36
# speedup vs baseline: 1.2095x; 1.2095x over previous
"""Trainium2 Bass kernel for nn_GAT_NFM (2x GAT encoder layers + NFM bilinear
pooling + projection) on 8 NeuronCores.

Sharding: nodes are partitioned contiguously across the 8 cores (N/8 each);
edges are partitioned by src node (the segment/aggregation axis). Each core
computes its shard of the per-layer transformed features; the shards are
all-gathered into a full per-core bf16 feature table in HBM; each core then
gathers its edges' dst rows from that table (dma_gather) and scatter-adds
w-weighted dst rows per src node with a w-weighted one-hot matmul on the
TensorEngine: out[i] = (sum_e w_e*Hw[dst_e]) / (sum_e w_e), where
w = exp(sigmoid(edge_val * (f1[src] + f2[dst]))) (the segment-max in the
reference softmax cancels algebraically).

Layer 1: f1/f2 depend only on the input x, so the per-edge weights w0 AND
the per-src softmax denominators are precomputed on the host and streamed
as dense per-slot metadata ([128, TOT] columns); the layer-1 table row is
exactly Hw0 (256 bf16 = 512B, the dma_gather elem quantum). The layer-1
output is normalized, transposed and immediately pushed through
W1ext = [W1 | W1@v11 | W1@v10] to build the layer-2 table (no HBM roundtrip
of H1).  Layer 2: table row = [Hw1 (128) | f2' | 1 | pad] (512B; the ones
column folds the denominator into the aggregation matmul); f1' comes from a
core-local 256B-row scalar table gathered by local src index in a prepass
ordered so its first ~K0 tiles overlap the T1 all-gather; and
exp(sigmoid(x)) is computed as exp(0.5*tanh(0.5x)+0.5) so both activations
live in one ACT table set (no per-tile table reloads).  Edges are grouped
(core, src-tile, dst-bucket) — buckets keep dma_gather indices within
int16 — sorted by dst inside each group for HBM page locality; pad slots
use index 0 and are masked by srel=-1/w=0 in the one-hot, and per-call
num_idxs is trimmed to the max real count over cores (MX16).  NFM is
computed while the T0 all-gather runs and the final projection is fused
into the layer-2 output stage.  PSUM->SBUF copies ride the otherwise-idle
ACT engine.  All-gathers are NOT split: neuronxcc requires contiguous
CollectiveCompute outputs.
"""

import math
import os

import numpy as np

import concourse.bass as bass
import concourse.bacc as bacc
import concourse.mybir as mybir
import concourse.tile as tile
from concourse.bass_utils import run_bass_kernel_spmd
from concourse.masks import make_identity

P = 128
N_CORES = 8
F32 = mybir.dt.float32
BF16 = mybir.dt.bfloat16
I32 = mybir.dt.int32
I16 = mybir.dt.int16
AF = mybir.ActivationFunctionType
OP = mybir.AluOpType


# ----------------------------------------------------------------- host prep

def _prep(inputs, n_cores=N_CORES, bucket_cap=25000):
    x = np.ascontiguousarray(np.asarray(inputs["x"], dtype=np.float32))
    ev = np.asarray(inputs["edge_val"], dtype=np.float32)
    src = np.asarray(inputs["edge_src"], dtype=np.int64)
    dst = np.asarray(inputs["edge_dst"], dtype=np.int64)
    W0 = np.asarray(inputs["W0"], dtype=np.float32)
    W1 = np.asarray(inputs["W1"], dtype=np.float32)
    v00 = np.asarray(inputs["v0_0"], dtype=np.float32)
    v01 = np.asarray(inputs["v0_1"], dtype=np.float32)
    v10 = np.asarray(inputs["v1_0"], dtype=np.float32)
    v11 = np.asarray(inputs["v1_1"], dtype=np.float32)
    fme = np.asarray(inputs["fm_emb"], dtype=np.float32)
    pjw = np.asarray(inputs["proj_W"], dtype=np.float32)
    pjb = np.asarray(inputs["proj_b"], dtype=np.float32)

    N, Din = x.shape
    E = src.shape[0]
    D0 = W0.shape[1]          # 256
    D1 = W1.shape[1]          # 128
    FM = fme.shape[1]         # 64
    NCLS = pjw.shape[1]       # 64
    assert N % n_cores == 0
    NSH = N // n_cores
    NT = math.ceil(NSH / P)
    assert NSH < (1 << 15), "local scalar-gather index must fit int16"

    # layer-1 attention is a pure function of x: precompute per-edge weights
    # w0 = exp(sigmoid(ev*(f1[src]+f2[dst]))) and per-src denominators.
    f1 = x @ (W0 @ v00)[:, 0]
    f2 = x @ (W0 @ v01)[:, 0]
    lg = ev * (f1[src] + f2[dst])
    w0 = np.exp(1.0 / (1.0 + np.exp(-lg))).astype(np.float32)
    den0 = np.bincount(src, weights=w0.astype(np.float64), minlength=N)
    rec0_n = (1.0 / np.maximum(den0, 1e-30)).astype(np.float32)

    # feature-table row widths (bf16 elements; 512B gather quantum)
    C0 = D0                   # 256: [Hw0]
    C1 = 2 * D1               # 256: [Hw1 | f2' | 1 | pad]
    CS = 128                  # local scalar-table row (256B)

    # dst buckets (int16 index range)
    NB = max(1, math.ceil(N / min(bucket_cap, 32000)))
    BSZ = math.ceil(N / NB)

    # ---- edge grouping: (core, src-tile, dst-bucket), dst-sorted in group
    loc = src % NSH
    core_of = src // NSH
    ltile = loc // P
    buck = dst // BSZ
    gid = (core_of * NT + ltile) * NB + buck
    order = np.argsort(gid * np.int64(N) + dst, kind="stable")
    sdst = dst[order]
    sgid = gid[order]
    sloc = loc[order]
    sev = ev[order]
    sw0 = w0[order]

    cnt = np.bincount(sgid, minlength=n_cores * NT * NB)
    cntc = cnt.reshape(n_cores, NT, NB)
    SZ = np.maximum(P, ((cntc.max(axis=0) + P - 1) // P) * P)  # [NT, NB]
    TPT = (SZ.sum(axis=1) // P).astype(np.int64)               # [NT]
    CUM = np.zeros(NT + 1, np.int64)
    CUM[1:] = np.cumsum(TPT)
    TOT = int(CUM[-1])                                         # cols per core
    TOTS = TOT * P                                             # slots per core
    OFF = np.zeros((NT, NB), np.int64)
    run = 0
    for nt in range(NT):
        for b in range(NB):
            OFF[nt, b] = run
            run += SZ[nt, b]
    assert run == TOTS

    grp = np.zeros(n_cores * NT * NB + 1, np.int64)
    grp[1:] = np.cumsum(cntc.reshape(-1))
    within = np.arange(E, dtype=np.int64) - grp[sgid]
    snt = (sgid // NB) % NT
    sb = sgid % NB
    pad_pos = OFF[snt, sb] + within

    # pad slots gather row 0 of their bucket (valid index; the one-hot masks
    # them via srel=-1/w=0). num_idxs per (tile, bucket) call is trimmed to
    # the max real count over cores, 16-aligned (MX16), so the 128-rounding
    # tail of each group is never gathered.
    MX16 = ((cntc.max(axis=0) + 15) // 16) * 16                # [NT, NB]
    dst16 = np.zeros((n_cores, TOTS), np.int16)
    src16 = np.zeros((n_cores, TOTS), np.int16)
    srel = np.full((n_cores, TOTS), -1.0, np.float32)
    w0s = np.zeros((n_cores, TOTS), np.float32)
    aval = np.zeros((n_cores, TOTS), np.float32)
    ci = core_of[order]
    dst16[ci, pad_pos] = (sdst - sb * BSZ).astype(np.int16)
    src16[ci, pad_pos] = sloc.astype(np.int16)
    srel[ci, pad_pos] = (sloc % P).astype(np.float32)
    w0s[ci, pad_pos] = sw0
    aval[ci, pad_pos] = sev

    def to_cols(a, dt):    # [TOTS] slot-major -> [P, TOT] (slot = col*128+p)
        return np.ascontiguousarray(a.reshape(TOT, P).T.astype(dt))

    def to_wrap16(a):      # [TOTS] -> [128, TOTS//16] 16-wrapped + replicated
        w = np.ascontiguousarray(a.reshape(TOTS // 16, 16).T)
        return np.ascontiguousarray(np.tile(w, (8, 1)))

    # tiny replicated weights
    w0e = np.ascontiguousarray(W0)                                # [Din, D0]
    w1e = np.ascontiguousarray(
        np.concatenate([W1, W1 @ v11, W1 @ v10], axis=1))         # [D0, D1+2]
    ee2 = np.ascontiguousarray(
        np.concatenate([fme, fme * fme], axis=1))                 # [Din, 2FM]
    pja = np.ascontiguousarray(pjw[:D1])                          # [D1, NCLS]
    pjbm = np.ascontiguousarray(0.5 * pjw[D1:])                   # [FM, NCLS]
    pbias = np.ascontiguousarray(pjb[None, :])                    # [1, NCLS]
    iota = np.broadcast_to(np.arange(P, dtype=np.float32), (P, P)).copy()

    import ml_dtypes

    def cast_bf16(a):
        return np.ascontiguousarray(a.astype(ml_dtypes.bfloat16))

    in_maps = []
    for c in range(n_cores):
        xt = np.ascontiguousarray(x[c * NSH:(c + 1) * NSH].T)     # [Din, NSH]
        rec0 = np.ones((P, NT), np.float32)
        rsh = rec0_n[c * NSH:(c + 1) * NSH]
        full = (NSH // P) * P
        rec0[:, :NSH // P] = rsh[:full].reshape(NSH // P, P).T
        if NSH % P:
            rec0[:NSH % P, NSH // P] = rsh[full:]
        in_maps.append({
            "xt": cast_bf16(xt),
            "idxf": to_wrap16(dst16[c]),
            "idxs": to_wrap16(src16[c]),
            "srel": to_cols(srel[c], np.float32),
            "w0s": to_cols(w0s[c], np.float32),
            "aval": cast_bf16(to_cols(aval[c], np.float32)),
            "rec0": rec0,
            "w0e": cast_bf16(w0e), "w1e": w1e, "ee2": cast_bf16(ee2),
            "pja": pja, "pjb": pjbm, "pbias": pbias,
            "iota": iota,
        })

    cfg = dict(N=N, E=E, Din=Din, D0=D0, D1=D1, FM=FM, NCLS=NCLS,
               NSH=NSH, NT=NT, NB=NB, BSZ=BSZ,
               SZ=[[int(v) for v in row] for row in SZ],
               OFF=[[int(v) for v in row] for row in OFF],
               TPT=[int(t) for t in TPT], CUM=[int(c) for c in CUM],
               TOT=TOT, C0=C0, C1=C1, CS=CS, n_cores=n_cores,
               MX=[[int(v) for v in row] for row in MX16])
    return cfg, in_maps


# ------------------------------------------------------------ device program

def _build(cfg, reps=1):
    N = cfg["N"]; Din = cfg["Din"]; D0 = cfg["D0"]; D1 = cfg["D1"]
    FM = cfg["FM"]; NCLS = cfg["NCLS"]; NSH = cfg["NSH"]; NT = cfg["NT"]
    NB = cfg["NB"]; BSZ = cfg["BSZ"]; SZ = cfg["SZ"]; OFF = cfg["OFF"]
    TPT = cfg["TPT"]; CUM = cfg["CUM"]; TOT = cfg["TOT"]; MX = cfg["MX"]
    C0 = cfg["C0"]; C1 = cfg["C1"]; CS = cfg["CS"]; n_cores = cfg["n_cores"]
    TPTmax = max(TPT)
    KD = Din // P             # 4
    KD0 = D0 // P             # 2
    FM2 = 2 * FM

    CH = int(os.environ.get("KCHUNK", "4096"))
    kfp8 = int(os.environ.get("KFP8", "0"))   # 0=bf16, 1=L2 fp8, 2=both fp8
    FP8 = mybir.dt.float8e4
    DT0 = FP8 if kfp8 >= 2 else BF16          # layer-1 table dtype
    DT1 = FP8 if kfp8 >= 1 else BF16          # layer-2 table dtype
    nqueues = int(os.environ.get("KQUEUES", "1"))
    # NOTE: neuronxcc's BIR verifier requires CollectiveCompute outputs to be
    # contiguous, so the all-gathers cannot be row-split (strided outputs).
    shared = os.environ.get("KSHARED", "0") == "1"
    agsplit = not shared and os.environ.get("KAGSPLIT", "0") == "1"
    nc = bacc.Bacc("TRN2", target_bir_lowering=False, debug=False,
                   num_devices=n_cores, num_swdge_queues=nqueues)
    qrr = [0]

    xt_d = nc.dram_tensor("xt", [Din, NSH], BF16, kind="ExternalInput")
    idxf_d = nc.dram_tensor("idxf", [P, TOT * 8], I16, kind="ExternalInput")
    idxs_d = nc.dram_tensor("idxs", [P, TOT * 8], I16, kind="ExternalInput")
    srel_d = nc.dram_tensor("srel", [P, TOT], F32, kind="ExternalInput")
    w0s_d = nc.dram_tensor("w0s", [P, TOT], F32, kind="ExternalInput")
    aval_d = nc.dram_tensor("aval", [P, TOT], BF16, kind="ExternalInput")
    rec0_d = nc.dram_tensor("rec0", [P, NT], F32, kind="ExternalInput")
    w0e_d = nc.dram_tensor("w0e", [Din, D0], BF16, kind="ExternalInput")
    w1e_d = nc.dram_tensor("w1e", [D0, D1 + 2], F32, kind="ExternalInput")
    ee2_d = nc.dram_tensor("ee2", [Din, FM2], BF16, kind="ExternalInput")
    pja_d = nc.dram_tensor("pja", [D1, NCLS], F32, kind="ExternalInput")
    pjb_d = nc.dram_tensor("pjb", [FM, NCLS], F32, kind="ExternalInput")
    pbias_d = nc.dram_tensor("pbias", [1, NCLS], F32, kind="ExternalInput")
    iota_d = nc.dram_tensor("iota", [P, P], F32, kind="ExternalInput")
    out_d = nc.dram_tensor("out", [NSH, NCLS], F32, kind="ExternalOutput")

    def tw(nt):
        return min(P, NSH - nt * P)

    HALF = (NSH // 2 // P) * P            # AG split row boundary (tile-align)
    half_tile = HALF // P - 1             # last tile fully inside first half

    with tile.TileContext(nc) as tc:
        with tc.tile_pool(name="dram", bufs=1, space="DRAM") as dram, \
             tc.tile_pool(name="const", bufs=1) as cpool, \
             tc.tile_pool(name="meta", bufs=1) as mpool:

            aspace = "Shared" if shared else "Local"
            T0L = dram.tile([NSH, C0], DT0)
            T0F = dram.tile([n_cores, NSH, C0], DT0, addr_space=aspace)
            T1L = dram.tile([NSH, C1], DT1)
            T1F = dram.tile([n_cores, NSH, C1], DT1, addr_space=aspace)
            T1S = dram.tile([NSH, CS], BF16)
            NFMT = dram.tile([FM, NSH], F32)

            # constants
            iota_t = cpool.tile([P, P], F32)
            nc.sync.dma_start(out=iota_t[:], in_=iota_d[:, :])
            ident = cpool.tile([P, P], F32)
            make_identity(nc, ident[:])
            ones_row = cpool.tile([1, P], F32)
            nc.vector.memset(ones_row[:], 1.0)
            half_col = cpool.tile([P, 1], F32)
            nc.vector.memset(half_col[:], 0.5)
            w0e_t = [cpool.tile([P, D0], BF16, tag=f"w0e{k}", name=f"w0e{k}")
                     for k in range(KD)]
            for k in range(KD):
                nc.sync.dma_start(out=w0e_t[k][:], in_=w0e_d[k * P:(k + 1) * P, :])
            w1e_t = [cpool.tile([P, D1 + 2], F32, tag=f"w1e{k}", name=f"w1e{k}")
                     for k in range(KD0)]
            for k in range(KD0):
                nc.sync.dma_start(out=w1e_t[k][:], in_=w1e_d[k * P:(k + 1) * P, :])
            ee2_t = [cpool.tile([P, FM2], BF16, tag=f"ee2{k}", name=f"ee2{k}")
                     for k in range(KD)]
            for k in range(KD):
                nc.sync.dma_start(out=ee2_t[k][:], in_=ee2_d[k * P:(k + 1) * P, :])
            pja_t = cpool.tile([D1, NCLS], F32)
            nc.sync.dma_start(out=pja_t[:], in_=pja_d[:, :])
            pjb_t = cpool.tile([FM, NCLS], F32)
            nc.sync.dma_start(out=pjb_t[:], in_=pjb_d[:, :])
            pbias_t = cpool.tile([1, NCLS], F32)
            nc.sync.dma_start(out=pbias_t[:], in_=pbias_d[:, :])

            # per-slot metadata, resident for the whole run
            srel_t = mpool.tile([P, TOT], F32)
            w0s_t = mpool.tile([P, TOT], F32)
            aval_t = mpool.tile([P, TOT], BF16)
            rec0_t = mpool.tile([P, NT], F32)
            nc.sync.dma_start(out=srel_t[:], in_=srel_d[:, :])
            nc.sync.dma_start(out=w0s_t[:], in_=w0s_d[:, :])
            nc.sync.dma_start(out=aval_t[:], in_=aval_d[:, :])
            nc.sync.dma_start(out=rec0_t[:], in_=rec0_d[:, :])
            # layer-2 f1'[src] per slot, filled by the scalar-gather prepass
            # that overlaps the T1 all-gather
            f1all = mpool.tile([P, TOT], F32)

            def ag(inp, outp, label):
                nc.gpsimd.collective_compute(
                    "AllGather", OP.bypass,
                    replica_groups=[list(range(n_cores))],
                    ins=[inp.opt()], outs=[outp.opt()])

            def _body():
                # -------- phase A: T0 rows = x @ W0 (bf16); NFM into SBUF
                with tc.tile_pool(name="a_sb", bufs=3) as asb, \
                     tc.tile_pool(name="a_xt", bufs=2) as axt, \
                     tc.tile_pool(name="a_ps", bufs=2, space="PSUM") as aps, \
                     tc.tile_pool(name="a_nf", bufs=2, space="PSUM") as anf:
                    for jc in range(0, NT, 4):
                        tiles = list(range(jc, min(jc + 4, NT)))
                        n0 = jc * P
                        cw = sum(tw(t) for t in tiles)
                        xtm = axt.tile([P, KD, 4 * P], BF16, tag="xt")
                        nc.sync.dma_start(
                            out=xtm[:, :, :cw],
                            in_=xt_d[:, n0:n0 + cw].rearrange(
                                "(k p) c -> p k c", p=P))
                        off = 0
                        for t in tiles:
                            wm = tw(t)
                            ap_ = aps.tile([P, D0], F32, tag="aps", space="PSUM")
                            for k in range(KD):
                                nc.tensor.matmul(out=ap_[:wm, :],
                                                 lhsT=xtm[:, k, off:off + wm],
                                                 rhs=w0e_t[k][:],
                                                 start=(k == 0), stop=(k == KD - 1))
                            st = asb.tile([P, C0], DT0, tag="st")
                            nc.scalar.copy(out=st[:wm, :], in_=ap_[:wm, :])
                            nc.sync.dma_start(out=T0L[t * P:t * P + wm, :],
                                              in_=st[:wm, :])
                            off += wm
                        if agsplit and tiles[0] <= half_tile < tiles[-1] + 1:
                            ag(T0L[0:HALF, :], T0F[:, 0:HALF, :], "t0a")
                    if agsplit:
                        ag(T0L[HALF:NSH, :], T0F[:, HALF:NSH, :], "t0b")
                    else:
                        ag(T0L[:, :], T0F[:, :, :], "t0")
                    # NFM (independent of the tables) computed while the T0
                    # all-gather runs on the collective cores.
                    for jc in range(0, NT, 4):
                        tiles = list(range(jc, min(jc + 4, NT)))
                        n0 = jc * P
                        cw = sum(tw(t) for t in tiles)
                        xtm = axt.tile([P, KD, 4 * P], BF16, tag="xt")
                        nc.sync.dma_start(
                            out=xtm[:, :, :cw],
                            in_=xt_d[:, n0:n0 + cw].rearrange(
                                "(k p) c -> p k c", p=P))
                        nf1 = anf.tile([FM, 4 * P], F32, tag="nf1", space="PSUM")
                        nf2 = anf.tile([FM, 4 * P], F32, tag="nf2", space="PSUM")
                        for k in range(KD):
                            nc.tensor.matmul(out=nf1[:, :cw], lhsT=ee2_t[k][:, :FM],
                                             rhs=xtm[:, k, :cw],
                                             start=(k == 0), stop=(k == KD - 1))
                        for k in range(KD):
                            nc.tensor.matmul(out=nf2[:, :cw],
                                             lhsT=ee2_t[k][:, FM:FM2],
                                             rhs=xtm[:, k, :cw],
                                             start=(k == 0), stop=(k == KD - 1))
                        s1 = asb.tile([FM, 4 * P], F32, tag="nfs1")
                        nc.vector.tensor_copy(out=s1[:, :cw], in_=nf1[:, :cw])
                        nfo = asb.tile([FM, 4 * P], F32, tag="nfo")
                        nc.vector.tensor_tensor(out=nfo[:, :cw], in0=s1[:, :cw],
                                                in1=s1[:, :cw], op=OP.mult)
                        nfm = asb.tile([FM, 4 * P], F32, tag="nfm")
                        nc.vector.tensor_tensor(out=nfm[:, :cw],
                                                in0=nfo[:, :cw],
                                                in1=nf2[:, :cw], op=OP.subtract)
                        nc.sync.dma_start(out=NFMT[:, n0:n0 + cw],
                                          in_=nfm[:, :cw])

                # per-bucket rank-slab views: bucket b = ranks [b*rpb, (b+1)*rpb)
                assert BSZ % NSH == 0
                rpb = BSZ // NSH
                T0Fb = [T0F[b * rpb:(b + 1) * rpb, :, :]
                        .rearrange("r n c -> (r n) c") for b in range(NB)]
                T1Fb = [T1F[b * rpb:(b + 1) * rpb, :, :]
                        .rearrange("r n c -> (r n) c") for b in range(NB)]

                # -------- L1 edge phase + fused T1 build
                with tc.tile_pool(name="e_g", bufs=3) as gp, \
                     tc.tile_pool(name="e_ix", bufs=3) as ixp, \
                     tc.tile_pool(name="e_oh", bufs=4) as ohp, \
                     tc.tile_pool(name="e_ps", bufs=2, space="PSUM") as psp, \
                     tc.tile_pool(name="e_tp", bufs=2, space="PSUM") as tpp, \
                     tc.tile_pool(name="e_bp", bufs=2, space="PSUM") as bpp, \
                     tc.tile_pool(name="e_sb", bufs=3) as esb:
                    # zero the gather buffers once: pad slots are skipped by
                    # the gather (idx=-1) and must never hold non-finite bits
                    # (0 * NaN = NaN in the aggregation matmul).
                    for zi in range(3):
                        z = gp.tile([P, TPTmax, C0], DT0, tag="g",
                                    name=f"zg1_{zi}")
                        nc.vector.memset(z[:], 0.0)
                    for nt in range(NT):
                        tpt = TPT[nt]
                        c0 = CUM[nt]
                        wm = tw(nt)
                        g = gp.tile([P, TPTmax, C0], DT0, tag="g")
                        ixf = ixp.tile([P, TPTmax * 8], I16, tag="ixf")
                        nc.sync.dma_start(
                            out=ixf[:, :tpt * 8],
                            in_=idxf_d[:, c0 * 8:(c0 + tpt) * 8])
                        for b in range(NB):
                            for z0 in range(0, MX[nt][b], CH):
                                sz = min(CH, MX[nt][b] - z0)
                                o8 = (OFF[nt][b] - CUM[nt] * P + z0) // 16
                                ot = (OFF[nt][b] - CUM[nt] * P + z0) // P
                                nc.gpsimd.dma_gather(
                                    out_ap=g[:, ot:ot + (sz + P - 1) // P, :],
                                    in_ap=T0Fb[b],
                                    idxs_ap=ixf[:, o8:o8 + (sz + 15) // 16],
                                    num_idxs=sz, num_idxs_reg=sz,
                                    elem_size=C0, elem_step=C0,
                                    single_packet=False,
                                    queue_num=qrr[0] % nqueues)
                                qrr[0] += 1
                        # aggregate with w-weighted one-hots
                        ps = psp.tile([P, D0], F32, tag="ps", space="PSUM")
                        for t in range(tpt):
                            oh = ohp.tile([P, P], DT0, tag="oh")
                            nc.vector.tensor_scalar(
                                out=oh[:], in0=iota_t[:],
                                scalar1=srel_t[:, c0 + t:c0 + t + 1],
                                scalar2=w0s_t[:, c0 + t:c0 + t + 1],
                                op0=OP.is_equal, op1=OP.mult)
                            nc.tensor.matmul(out=ps[:], lhsT=oh[:],
                                             rhs=g[:, t, :],
                                             start=(t == 0), stop=(t == tpt - 1))
                        hsb = esb.tile([P, D0], F32, tag="hsb")
                        nc.scalar.activation(hsb[:], ps[:], AF.Copy,
                                             scale=rec0_t[:, nt:nt + 1])
                        # fused: T1 row = [H1 @ W1 | f2' | 1 | pad], T1S = f1'
                        bp = bpp.tile([P, D1 + 2], F32, tag="bp", space="PSUM")
                        for k in range(KD0):
                            tp = tpp.tile([P, P], F32, tag="tp", space="PSUM")
                            nc.tensor.transpose(out=tp[:, :wm],
                                                in_=hsb[:wm, k * P:(k + 1) * P],
                                                identity=ident[:wm, :wm])
                            ht = esb.tile([P, P], F32, tag="ht")
                            nc.scalar.copy(out=ht[:, :wm], in_=tp[:, :wm])
                            nc.tensor.matmul(out=bp[:wm, :], lhsT=ht[:, :wm],
                                             rhs=w1e_t[k][:],
                                             start=(k == 0), stop=(k == KD0 - 1))
                        st = esb.tile([P, C1], DT1, tag="st2")
                        nc.scalar.copy(out=st[:wm, 0:D1 + 1],
                                       in_=bp[:wm, 0:D1 + 1])
                        nc.vector.memset(st[:, D1 + 1:D1 + 2], 1.0)
                        nc.vector.memset(st[:, D1 + 2:C1], 0.0)
                        nc.sync.dma_start(out=T1L[nt * P:nt * P + wm, :],
                                          in_=st[:wm, :])
                        sc = esb.tile([P, CS], BF16, tag="sc2")
                        nc.vector.memset(sc[:, :], 0.0)
                        nc.scalar.copy(out=sc[:wm, 0:1],
                                       in_=bp[:wm, D1 + 1:D1 + 2])
                        nc.sync.dma_start(out=T1S[nt * P:nt * P + wm, :],
                                          in_=sc[:wm, :])
                        if agsplit and nt == half_tile:
                            ag(T1L[0:HALF, :], T1F[:, 0:HALF, :], "t1a")
                    if agsplit:
                        ag(T1L[HALF:NSH, :], T1F[:, HALF:NSH, :], "t1b")
                    else:
                        ag(T1L[:, :], T1F[:, :, :], "t1")

                # -------- L2 edge phase + fused projection
                with tc.tile_pool(name="f_g", bufs=3) as gp2, \
                     tc.tile_pool(name="p_gs", bufs=2) as gsp, \
                     tc.tile_pool(name="p_ix", bufs=2) as ixps, \
                     tc.tile_pool(name="f_ix", bufs=3) as ixp2, \
                     tc.tile_pool(name="f_w", bufs=2) as wp, \
                     tc.tile_pool(name="f_oh", bufs=4) as ohp2, \
                     tc.tile_pool(name="f_ps", bufs=2, space="PSUM") as psp2, \
                     tc.tile_pool(name="f_tp", bufs=2, space="PSUM") as tpp2, \
                     tc.tile_pool(name="f_fp", bufs=2, space="PSUM") as cfp, \
                     tc.tile_pool(name="f_sb", bufs=3) as esb2:

                    def gs_pre(nt):
                        # f1'[src] from the core-local scalar table into
                        # f1all. Depends only on T1S (ready mid-L1), so the
                        # leading K0 tiles overlap the T1 all-gather.
                        tpt = TPT[nt]
                        c0 = CUM[nt]
                        gs = gsp.tile([P, TPTmax, CS], BF16, tag="gs",
                                      name="gs")
                        ixs = ixps.tile([P, TPTmax * 8], I16, tag="ixs",
                                        name="ixs")
                        nc.sync.dma_start(
                            out=ixs[:, :tpt * 8],
                            in_=idxs_d[:, c0 * 8:(c0 + tpt) * 8])
                        for q0 in range(0, tpt, 32):
                            qn = min(32, tpt - q0)
                            nc.gpsimd.dma_gather(
                                out_ap=gs[:, q0:q0 + qn, :], in_ap=T1S[:, :],
                                idxs_ap=ixs[:, q0 * 8:(q0 + qn) * 8],
                                num_idxs=qn * P, num_idxs_reg=qn * P,
                                elem_size=CS, single_packet=False,
                                queue_num=qrr[0] % nqueues)
                            qrr[0] += 1
                        nc.scalar.copy(out=f1all[:, c0:c0 + tpt],
                                       in_=gs[:, :tpt, 0])

                    # one-time zero of the gather pools so pad slots can never
                    # hold non-finite garbage (reused buffers stay finite).
                    for zi in range(3):
                        z = gp2.tile([P, TPTmax, C1], DT1, tag="g2",
                                     name=f"zg2_{zi}")
                        nc.vector.memset(z[:], 0.0)
                    for zi in range(2):
                        z2 = gsp.tile([P, TPTmax, CS], BF16, tag="gs",
                                      name=f"zgs_{zi}")
                        nc.vector.memset(z2[:], 0.0)
                    K0 = min(NT, int(os.environ.get("KPRE", "56")))
                    for nt in range(K0):
                        gs_pre(nt)
                    for nt in range(NT):
                        if K0 + nt < NT:
                            gs_pre(K0 + nt)
                        tpt = TPT[nt]
                        c0 = CUM[nt]
                        wm = tw(nt)
                        g = gp2.tile([P, TPTmax, C1], DT1, tag="g2")
                        ixf = ixp2.tile([P, TPTmax * 8], I16, tag="ixf2")
                        nc.sync.dma_start(
                            out=ixf[:, :tpt * 8],
                            in_=idxf_d[:, c0 * 8:(c0 + tpt) * 8])
                        for b in range(NB):
                            for z0 in range(0, MX[nt][b], CH):
                                sz = min(CH, MX[nt][b] - z0)
                                o8 = (OFF[nt][b] - CUM[nt] * P + z0) // 16
                                ot = (OFF[nt][b] - CUM[nt] * P + z0) // P
                                nc.gpsimd.dma_gather(
                                    out_ap=g[:, ot:ot + (sz + P - 1) // P, :],
                                    in_ap=T1Fb[b],
                                    idxs_ap=ixf[:, o8:o8 + (sz + 15) // 16],
                                    num_idxs=sz, num_idxs_reg=sz,
                                    elem_size=C1, elem_step=C1,
                                    single_packet=False,
                                    queue_num=qrr[0] % nqueues)
                                qrr[0] += 1
                        # w = exp(sigmoid(aval*(f1+f2))) = exp(.5*tanh(.5x)+.5)
                        w = wp.tile([P, TPTmax], F32, tag="w")
                        nc.vector.tensor_tensor(out=w[:, :tpt],
                                                in0=f1all[:, c0:c0 + tpt],
                                                in1=g[:, :tpt, D1], op=OP.add)
                        nc.vector.tensor_tensor(out=w[:, :tpt], in0=w[:, :tpt],
                                                in1=aval_t[:, c0:c0 + tpt],
                                                op=OP.mult)
                        nc.scalar.activation(w[:, :tpt], w[:, :tpt], AF.Tanh,
                                             scale=0.5)
                        nc.scalar.activation(w[:, :tpt], w[:, :tpt], AF.Exp,
                                             scale=0.5, bias=half_col[:, :1])
                        ps = psp2.tile([P, D1 + 2], F32, tag="ps2", space="PSUM")
                        for t in range(tpt):
                            oh = ohp2.tile([P, P], DT1, tag="oh2")
                            nc.vector.tensor_scalar(
                                out=oh[:], in0=iota_t[:],
                                scalar1=srel_t[:, c0 + t:c0 + t + 1],
                                scalar2=w[:, t:t + 1],
                                op0=OP.is_equal, op1=OP.mult)
                            nc.tensor.matmul(out=ps[:], lhsT=oh[:],
                                             rhs=g[:, t, 0:D1 + 2],
                                             start=(t == 0), stop=(t == tpt - 1))
                        den = esb2.tile([P, 1], F32, tag="den")
                        nc.vector.tensor_scalar(out=den[:], in0=ps[:, D1 + 1:D1 + 2],
                                                scalar1=1e-30, scalar2=None,
                                                op0=OP.add)
                        rec = esb2.tile([P, 1], F32, tag="rec")
                        nc.vector.reciprocal(rec[:], den[:])
                        hsb = esb2.tile([P, D1], F32, tag="hsb2")
                        nc.scalar.activation(hsb[:], ps[:, 0:D1], AF.Copy,
                                             scale=rec[:, :1])
                        # fused projection: out = [H2 | nfm] @ proj + b
                        n0 = nt * P
                        tp = tpp2.tile([P, P], F32, tag="tp2", space="PSUM")
                        nc.tensor.transpose(out=tp[:, :wm], in_=hsb[:wm, 0:D1],
                                            identity=ident[:wm, :wm])
                        h2t = esb2.tile([P, P], F32, tag="h2t")
                        nc.scalar.copy(out=h2t[:, :wm], in_=tp[:, :wm])
                        nft = esb2.tile([FM, P], F32, tag="nft")
                        nc.sync.dma_start(out=nft[:, :wm], in_=NFMT[:, n0:n0 + wm])
                        fps = cfp.tile([P, NCLS], F32, tag="fps", space="PSUM")
                        nc.tensor.matmul(out=fps[:wm, :], lhsT=h2t[:, :wm],
                                         rhs=pja_t[:], start=True, stop=False)
                        nc.tensor.matmul(out=fps[:wm, :],
                                         lhsT=nft[:, :wm],
                                         rhs=pjb_t[:], start=False, stop=False)
                        nc.tensor.matmul(out=fps[:wm, :], lhsT=ones_row[:1, :wm],
                                         rhs=pbias_t[:], start=False, stop=True)
                        ot2 = esb2.tile([P, NCLS], F32, tag="ot")
                        nc.scalar.copy(out=ot2[:wm, :], in_=fps[:wm, :])
                        nc.sync.dma_start(out=out_d[n0:n0 + wm, :], in_=ot2[:wm, :])

            for _rep in range(reps):
                _body()

    nc.finalize()
    return nc


_CACHE = {}


def _get_program(cfg_key, cfg):
    if cfg_key not in _CACHE:
        _CACHE[cfg_key] = _build(cfg)
    return _CACHE[cfg_key]


def kernel(**inputs) -> np.ndarray:
    cfg, in_maps = _prep(inputs)
    cfg_key = (cfg["N"], cfg["E"], cfg["Din"], cfg["D0"], cfg["D1"],
               cfg["FM"], cfg["NCLS"], tuple(cfg["TPT"]),
               tuple(tuple(r) for r in cfg["SZ"]))
    nc = _get_program(cfg_key, cfg)
    res = run_bass_kernel_spmd(nc, in_maps, list(range(cfg["n_cores"])))
    out = np.concatenate(
        [res.results[c]["out"] for c in range(cfg["n_cores"])], axis=0)
    return out.astype(np.float32)


# revision 41
# speedup vs baseline: 1.3604x; 1.1247x over previous
"""Trainium2 Bass kernel for nn_GAT_NFM (2x GAT encoder layers + NFM bilinear
pooling + projection) on 8 NeuronCores.

Sharding: nodes are partitioned contiguously across the 8 cores (N/8 each);
edges are partitioned by src node (the segment/aggregation axis). Each core
computes its shard of the per-layer transformed features; the shards are
all-gathered into a full per-core bf16 feature table in HBM; each core then
gathers its edges' dst rows from that table (dma_gather) and scatter-adds
w-weighted dst rows per src node with a w-weighted one-hot matmul on the
TensorEngine: out[i] = (sum_e w_e*Hw[dst_e]) / (sum_e w_e), where
w = exp(sigmoid(edge_val * (f1[src] + f2[dst]))) (the segment-max in the
reference softmax cancels algebraically).

Layer 1: f1/f2 depend only on the input x, so the per-edge weights w0 AND
the per-src softmax denominators are precomputed on the host and streamed
as dense per-slot metadata ([128, TOT] columns); the layer-1 table row is
exactly Hw0 (256 bf16 = 512B, the dma_gather elem quantum). The layer-1
output is normalized, transposed and immediately pushed through
W1ext = [W1 | W1@v11 | W1@v10] to build the layer-2 table (no HBM roundtrip
of H1).  Layer 2: table row = [Hw1 (128) | f2' | 1 | pad] (512B; the ones
column folds the denominator into the aggregation matmul); f1' comes from a
core-local 256B-row scalar table gathered by local src index in a prepass
ordered so its first ~K0 tiles overlap the T1 all-gather; and
exp(sigmoid(x)) is computed as exp(0.5*tanh(0.5x)+0.5) so both activations
live in one ACT table set (no per-tile table reloads).  Edges are grouped
(core, src-tile, dst-bucket) — buckets keep dma_gather indices within
int16 — sorted by dst inside each group for HBM page locality; pad slots
use index 0 and are masked by srel=-1/w=0 in the one-hot, and per-call
num_idxs is trimmed to the max real count over cores (MX16).  NFM is
computed while the T0 all-gather runs and the final projection is fused
into the layer-2 output stage.  PSUM->SBUF copies ride the otherwise-idle
ACT engine.  All-gathers are NOT split: neuronxcc requires contiguous
CollectiveCompute outputs.
"""

import math
import os

import numpy as np

import concourse.bass as bass
import concourse.bacc as bacc
import concourse.mybir as mybir
import concourse.tile as tile
from concourse.bass_utils import run_bass_kernel_spmd
from concourse.masks import make_identity

P = 128
N_CORES = 8
F32 = mybir.dt.float32
BF16 = mybir.dt.bfloat16
I32 = mybir.dt.int32
I16 = mybir.dt.int16
AF = mybir.ActivationFunctionType
OP = mybir.AluOpType


# ----------------------------------------------------------------- host prep

def _prep(inputs, n_cores=N_CORES, bucket_cap=25000):
    x = np.ascontiguousarray(np.asarray(inputs["x"], dtype=np.float32))
    ev = np.asarray(inputs["edge_val"], dtype=np.float32)
    src = np.asarray(inputs["edge_src"], dtype=np.int64)
    dst = np.asarray(inputs["edge_dst"], dtype=np.int64)
    W0 = np.asarray(inputs["W0"], dtype=np.float32)
    W1 = np.asarray(inputs["W1"], dtype=np.float32)
    v00 = np.asarray(inputs["v0_0"], dtype=np.float32)
    v01 = np.asarray(inputs["v0_1"], dtype=np.float32)
    v10 = np.asarray(inputs["v1_0"], dtype=np.float32)
    v11 = np.asarray(inputs["v1_1"], dtype=np.float32)
    fme = np.asarray(inputs["fm_emb"], dtype=np.float32)
    pjw = np.asarray(inputs["proj_W"], dtype=np.float32)
    pjb = np.asarray(inputs["proj_b"], dtype=np.float32)

    N, Din = x.shape
    E = src.shape[0]
    D0 = W0.shape[1]          # 256
    D1 = W1.shape[1]          # 128
    FM = fme.shape[1]         # 64
    NCLS = pjw.shape[1]       # 64
    assert N % n_cores == 0
    NSH = N // n_cores
    NT = math.ceil(NSH / P)
    assert NSH < (1 << 15), "local scalar-gather index must fit int16"

    # layer-1 attention is a pure function of x: precompute per-edge weights
    # w0 = exp(sigmoid(ev*(f1[src]+f2[dst]))) and per-src denominators.
    f1 = x @ (W0 @ v00)[:, 0]
    f2 = x @ (W0 @ v01)[:, 0]
    lg = ev * (f1[src] + f2[dst])
    w0 = np.exp(1.0 / (1.0 + np.exp(-lg))).astype(np.float32)
    den0 = np.bincount(src, weights=w0.astype(np.float64), minlength=N)
    rec0_n = (1.0 / np.maximum(den0, 1e-30)).astype(np.float32)

    # feature-table row widths (bf16 elements; 512B gather quantum)
    C0 = D0                   # 256: [Hw0]
    C1 = 2 * D1               # 256: [Hw1 | f2' | 1 | pad]
    CS = 128                  # local scalar-table row (256B)

    # dst buckets (int16 index range)
    NB = max(1, math.ceil(N / min(bucket_cap, 32000)))
    BSZ = math.ceil(N / NB)

    # ---- edge grouping: (core, src-tile, dst-bucket), dst-sorted in group
    loc = src % NSH
    core_of = src // NSH
    ltile = loc // P
    buck = dst // BSZ
    gid = (core_of * NT + ltile) * NB + buck
    order = np.argsort(gid * np.int64(N) + dst, kind="stable")
    sdst = dst[order]
    sgid = gid[order]
    sloc = loc[order]
    sev = ev[order]
    sw0 = w0[order]

    cnt = np.bincount(sgid, minlength=n_cores * NT * NB)
    cntc = cnt.reshape(n_cores, NT, NB)
    SZ = np.maximum(P, ((cntc.max(axis=0) + P - 1) // P) * P)  # [NT, NB]
    TPT = (SZ.sum(axis=1) // P).astype(np.int64)               # [NT]
    CUM = np.zeros(NT + 1, np.int64)
    CUM[1:] = np.cumsum(TPT)
    TOT = int(CUM[-1])                                         # cols per core
    TOTS = TOT * P                                             # slots per core
    OFF = np.zeros((NT, NB), np.int64)
    run = 0
    for nt in range(NT):
        for b in range(NB):
            OFF[nt, b] = run
            run += SZ[nt, b]
    assert run == TOTS

    grp = np.zeros(n_cores * NT * NB + 1, np.int64)
    grp[1:] = np.cumsum(cntc.reshape(-1))
    within = np.arange(E, dtype=np.int64) - grp[sgid]
    snt = (sgid // NB) % NT
    sb = sgid % NB
    pad_pos = OFF[snt, sb] + within

    # pad slots gather row 0 of their bucket (valid index; the one-hot masks
    # them via srel=-1/w=0). num_idxs per (tile, bucket) call is trimmed to
    # the max real count over cores, 16-aligned (MX16), so the 128-rounding
    # tail of each group is never gathered.
    MX16 = ((cntc.max(axis=0) + 15) // 16) * 16                # [NT, NB]
    dst16 = np.zeros((n_cores, TOTS), np.int16)
    src16 = np.zeros((n_cores, TOTS), np.int16)
    srel = np.full((n_cores, TOTS), -1.0, np.float32)
    w0s = np.zeros((n_cores, TOTS), np.float32)
    aval = np.zeros((n_cores, TOTS), np.float32)
    ci = core_of[order]
    dst16[ci, pad_pos] = (sdst - sb * BSZ).astype(np.int16)
    src16[ci, pad_pos] = sloc.astype(np.int16)
    srel[ci, pad_pos] = (sloc % P).astype(np.float32)
    w0s[ci, pad_pos] = sw0
    aval[ci, pad_pos] = sev

    def to_cols(a, dt):    # [TOTS] slot-major -> [P, TOT] (slot = col*128+p)
        return np.ascontiguousarray(a.reshape(TOT, P).T.astype(dt))

    def to_wrap16(a):      # [TOTS] -> [128, TOTS//16] 16-wrapped + replicated
        w = np.ascontiguousarray(a.reshape(TOTS // 16, 16).T)
        return np.ascontiguousarray(np.tile(w, (8, 1)))

    # tiny replicated weights
    w0e = np.ascontiguousarray(W0)                                # [Din, D0]
    w1e = np.ascontiguousarray(
        np.concatenate([W1, W1 @ v11, W1 @ v10], axis=1))         # [D0, D1+2]
    ee2 = np.ascontiguousarray(
        np.concatenate([fme, fme * fme], axis=1))                 # [Din, 2FM]
    pja = np.ascontiguousarray(pjw[:D1])                          # [D1, NCLS]
    pjbm = np.ascontiguousarray(0.5 * pjw[D1:])                   # [FM, NCLS]
    pbias = np.ascontiguousarray(pjb[None, :])                    # [1, NCLS]
    iota = np.broadcast_to(np.arange(P, dtype=np.float32), (P, P)).copy()

    import ml_dtypes

    def cast_bf16(a):
        return np.ascontiguousarray(a.astype(ml_dtypes.bfloat16))

    in_maps = []
    for c in range(n_cores):
        xt = np.ascontiguousarray(x[c * NSH:(c + 1) * NSH].T)     # [Din, NSH]
        rec0 = np.ones((P, NT), np.float32)
        rsh = rec0_n[c * NSH:(c + 1) * NSH]
        full = (NSH // P) * P
        rec0[:, :NSH // P] = rsh[:full].reshape(NSH // P, P).T
        if NSH % P:
            rec0[:NSH % P, NSH // P] = rsh[full:]
        in_maps.append({
            "xt": cast_bf16(xt),
            "idxf": to_wrap16(dst16[c]),
            "idxs": to_wrap16(src16[c]),
            "srel": to_cols(srel[c], np.float32),
            "w0s": to_cols(w0s[c], np.float32),
            "aval": cast_bf16(to_cols(aval[c], np.float32)),
            "rec0": rec0,
            "w0e": cast_bf16(w0e), "w1e": w1e, "ee2": cast_bf16(ee2),
            "pja": pja, "pjb": pjbm, "pbias": pbias,
            "iota": iota,
        })

    cfg = dict(N=N, E=E, Din=Din, D0=D0, D1=D1, FM=FM, NCLS=NCLS,
               NSH=NSH, NT=NT, NB=NB, BSZ=BSZ,
               SZ=[[int(v) for v in row] for row in SZ],
               OFF=[[int(v) for v in row] for row in OFF],
               TPT=[int(t) for t in TPT], CUM=[int(c) for c in CUM],
               TOT=TOT, C0=C0, C1=C1, CS=CS, n_cores=n_cores,
               MX=[[int(v) for v in row] for row in MX16])
    return cfg, in_maps


# ------------------------------------------------------------ device program

def _build(cfg, reps=1):
    N = cfg["N"]; Din = cfg["Din"]; D0 = cfg["D0"]; D1 = cfg["D1"]
    FM = cfg["FM"]; NCLS = cfg["NCLS"]; NSH = cfg["NSH"]; NT = cfg["NT"]
    NB = cfg["NB"]; BSZ = cfg["BSZ"]; SZ = cfg["SZ"]; OFF = cfg["OFF"]
    TPT = cfg["TPT"]; CUM = cfg["CUM"]; TOT = cfg["TOT"]; MX = cfg["MX"]
    C0 = cfg["C0"]; C1 = cfg["C1"]; CS = cfg["CS"]; n_cores = cfg["n_cores"]
    TPTmax = max(TPT)
    KD = Din // P             # 4
    KD0 = D0 // P             # 2
    FM2 = 2 * FM

    CH = int(os.environ.get("KCHUNK", "4096"))
    kfp8 = int(os.environ.get("KFP8", "0"))   # 0=bf16, 1=L2 fp8, 2=both fp8
    FP8 = mybir.dt.float8e4
    DT0 = FP8 if kfp8 >= 2 else BF16          # layer-1 table dtype
    DT1 = FP8 if kfp8 >= 1 else BF16          # layer-2 table dtype
    nqueues = int(os.environ.get("KQUEUES", "1"))
    # NOTE: neuronxcc's BIR verifier requires CollectiveCompute outputs to be
    # contiguous, so the all-gathers cannot be row-split (strided outputs).
    shared = os.environ.get("KSHARED", "0") == "1"
    agsplit = not shared and os.environ.get("KAGSPLIT", "0") == "1"
    nc = bacc.Bacc("TRN2", target_bir_lowering=False, debug=False,
                   num_devices=n_cores, num_swdge_queues=nqueues)
    qrr = [0]

    xt_d = nc.dram_tensor("xt", [Din, NSH], BF16, kind="ExternalInput")
    idxf_d = nc.dram_tensor("idxf", [P, TOT * 8], I16, kind="ExternalInput")
    idxs_d = nc.dram_tensor("idxs", [P, TOT * 8], I16, kind="ExternalInput")
    srel_d = nc.dram_tensor("srel", [P, TOT], F32, kind="ExternalInput")
    w0s_d = nc.dram_tensor("w0s", [P, TOT], F32, kind="ExternalInput")
    aval_d = nc.dram_tensor("aval", [P, TOT], BF16, kind="ExternalInput")
    rec0_d = nc.dram_tensor("rec0", [P, NT], F32, kind="ExternalInput")
    w0e_d = nc.dram_tensor("w0e", [Din, D0], BF16, kind="ExternalInput")
    w1e_d = nc.dram_tensor("w1e", [D0, D1 + 2], F32, kind="ExternalInput")
    ee2_d = nc.dram_tensor("ee2", [Din, FM2], BF16, kind="ExternalInput")
    pja_d = nc.dram_tensor("pja", [D1, NCLS], F32, kind="ExternalInput")
    pjb_d = nc.dram_tensor("pjb", [FM, NCLS], F32, kind="ExternalInput")
    pbias_d = nc.dram_tensor("pbias", [1, NCLS], F32, kind="ExternalInput")
    iota_d = nc.dram_tensor("iota", [P, P], F32, kind="ExternalInput")
    out_d = nc.dram_tensor("out", [NSH, NCLS], F32, kind="ExternalOutput")

    def tw(nt):
        return min(P, NSH - nt * P)

    HALF = (NSH // 2 // P) * P            # AG split row boundary (tile-align)
    half_tile = HALF // P - 1             # last tile fully inside first half

    with tile.TileContext(nc) as tc:
        with tc.tile_pool(name="dram", bufs=1, space="DRAM") as dram, \
             tc.tile_pool(name="const", bufs=1) as cpool, \
             tc.tile_pool(name="meta", bufs=1) as mpool:

            aspace = "Shared" if shared else "Local"
            T0L = dram.tile([NSH, C0], DT0)
            T0F = dram.tile([n_cores, NSH, C0], DT0, addr_space=aspace)
            T1L = dram.tile([NSH, C1], DT1)
            T1F = dram.tile([n_cores, NSH, C1], DT1, addr_space=aspace)
            T1S = dram.tile([NSH, CS], BF16)
            NFMT = dram.tile([FM, NSH], F32)

            # constants
            iota_t = cpool.tile([P, P], F32)
            nc.sync.dma_start(out=iota_t[:], in_=iota_d[:, :])
            ident = cpool.tile([P, P], F32)
            make_identity(nc, ident[:])
            ones_row = cpool.tile([1, P], F32)
            nc.vector.memset(ones_row[:], 1.0)
            half_col = cpool.tile([P, 1], F32)
            nc.vector.memset(half_col[:], 0.5)
            w0e_t = [cpool.tile([P, D0], BF16, tag=f"w0e{k}", name=f"w0e{k}")
                     for k in range(KD)]
            for k in range(KD):
                nc.sync.dma_start(out=w0e_t[k][:], in_=w0e_d[k * P:(k + 1) * P, :])
            w1e_t = [cpool.tile([P, D1 + 2], F32, tag=f"w1e{k}", name=f"w1e{k}")
                     for k in range(KD0)]
            for k in range(KD0):
                nc.sync.dma_start(out=w1e_t[k][:], in_=w1e_d[k * P:(k + 1) * P, :])
            ee2_t = [cpool.tile([P, FM2], BF16, tag=f"ee2{k}", name=f"ee2{k}")
                     for k in range(KD)]
            for k in range(KD):
                nc.sync.dma_start(out=ee2_t[k][:], in_=ee2_d[k * P:(k + 1) * P, :])
            pja_t = cpool.tile([D1, NCLS], F32)
            nc.sync.dma_start(out=pja_t[:], in_=pja_d[:, :])
            pjb_t = cpool.tile([FM, NCLS], F32)
            nc.sync.dma_start(out=pjb_t[:], in_=pjb_d[:, :])
            pbias_t = cpool.tile([1, NCLS], F32)
            nc.sync.dma_start(out=pbias_t[:], in_=pbias_d[:, :])

            # per-slot metadata, resident for the whole run
            srel_t = mpool.tile([P, TOT], F32)
            w0s_t = mpool.tile([P, TOT], F32)
            aval_t = mpool.tile([P, TOT], BF16)
            rec0_t = mpool.tile([P, NT], F32)
            nc.sync.dma_start(out=srel_t[:], in_=srel_d[:, :])
            nc.sync.dma_start(out=w0s_t[:], in_=w0s_d[:, :])
            nc.sync.dma_start(out=aval_t[:], in_=aval_d[:, :])
            nc.sync.dma_start(out=rec0_t[:], in_=rec0_d[:, :])
            # layer-2 f1'[src] per slot, filled by the scalar-gather prepass
            # that overlaps the T1 all-gather
            f1all = mpool.tile([P, TOT], F32)

            def ag(inp, outp, label):
                nc.gpsimd.collective_compute(
                    "AllGather", OP.bypass,
                    replica_groups=[list(range(n_cores))],
                    ins=[inp.opt()], outs=[outp.opt()])

            def _body():
                # -------- phase A: T0 rows = x @ W0 (bf16); NFM into SBUF
                with tc.tile_pool(name="a_sb", bufs=3) as asb, \
                     tc.tile_pool(name="a_xt", bufs=2) as axt, \
                     tc.tile_pool(name="a_ps", bufs=2, space="PSUM") as aps, \
                     tc.tile_pool(name="a_nf", bufs=2, space="PSUM") as anf:
                    for jc in range(0, NT, 4):
                        tiles = list(range(jc, min(jc + 4, NT)))
                        n0 = jc * P
                        cw = sum(tw(t) for t in tiles)
                        xtm = axt.tile([P, KD, 4 * P], BF16, tag="xt")
                        nc.sync.dma_start(
                            out=xtm[:, :, :cw],
                            in_=xt_d[:, n0:n0 + cw].rearrange(
                                "(k p) c -> p k c", p=P))
                        off = 0
                        full = all(tw(t) == P for t in tiles)
                        stm = asb.tile([P, 4, C0], DT0, tag="st",
                                       name="stm") if full else None
                        for ti, t in enumerate(tiles):
                            wm = tw(t)
                            ap_ = aps.tile([P, D0], F32, tag="aps", space="PSUM")
                            for k in range(KD):
                                nc.tensor.matmul(out=ap_[:wm, :],
                                                 lhsT=xtm[:, k, off:off + wm],
                                                 rhs=w0e_t[k][:],
                                                 start=(k == 0), stop=(k == KD - 1))
                            if full:
                                nc.scalar.copy(out=stm[:, ti, :], in_=ap_[:, :])
                            else:
                                st = asb.tile([P, C0], DT0, tag="st1")
                                nc.scalar.copy(out=st[:wm, :], in_=ap_[:wm, :])
                                nc.sync.dma_start(
                                    out=T0L[t * P:t * P + wm, :],
                                    in_=st[:wm, :])
                            off += wm
                        if full:
                            nc.sync.dma_start(
                                out=T0L[n0:n0 + cw, :].rearrange(
                                    "(k p) c -> p k c", p=P),
                                in_=stm[:, :len(tiles), :])
                        if agsplit and tiles[0] <= half_tile < tiles[-1] + 1:
                            ag(T0L[0:HALF, :], T0F[:, 0:HALF, :], "t0a")
                    if agsplit:
                        ag(T0L[HALF:NSH, :], T0F[:, HALF:NSH, :], "t0b")
                    else:
                        ag(T0L[:, :], T0F[:, :, :], "t0")
                    # NFM (independent of the tables) computed while the T0
                    # all-gather runs on the collective cores.
                    for jc in range(0, NT, 4):
                        tiles = list(range(jc, min(jc + 4, NT)))
                        n0 = jc * P
                        cw = sum(tw(t) for t in tiles)
                        xtm = axt.tile([P, KD, 4 * P], BF16, tag="xt")
                        nc.sync.dma_start(
                            out=xtm[:, :, :cw],
                            in_=xt_d[:, n0:n0 + cw].rearrange(
                                "(k p) c -> p k c", p=P))
                        nf1 = anf.tile([FM, 4 * P], F32, tag="nf1", space="PSUM")
                        nf2 = anf.tile([FM, 4 * P], F32, tag="nf2", space="PSUM")
                        for k in range(KD):
                            nc.tensor.matmul(out=nf1[:, :cw], lhsT=ee2_t[k][:, :FM],
                                             rhs=xtm[:, k, :cw],
                                             start=(k == 0), stop=(k == KD - 1))
                        for k in range(KD):
                            nc.tensor.matmul(out=nf2[:, :cw],
                                             lhsT=ee2_t[k][:, FM:FM2],
                                             rhs=xtm[:, k, :cw],
                                             start=(k == 0), stop=(k == KD - 1))
                        s1 = asb.tile([FM, 4 * P], F32, tag="nfs1")
                        nc.vector.tensor_copy(out=s1[:, :cw], in_=nf1[:, :cw])
                        nfo = asb.tile([FM, 4 * P], F32, tag="nfo")
                        nc.vector.tensor_tensor(out=nfo[:, :cw], in0=s1[:, :cw],
                                                in1=s1[:, :cw], op=OP.mult)
                        nfm = asb.tile([FM, 4 * P], F32, tag="nfm")
                        nc.vector.tensor_tensor(out=nfm[:, :cw],
                                                in0=nfo[:, :cw],
                                                in1=nf2[:, :cw], op=OP.subtract)
                        nc.sync.dma_start(out=NFMT[:, n0:n0 + cw],
                                          in_=nfm[:, :cw])

                # per-bucket rank-slab views: bucket b = ranks [b*rpb, (b+1)*rpb)
                assert BSZ % NSH == 0
                rpb = BSZ // NSH
                T0Fb = [T0F[b * rpb:(b + 1) * rpb, :, :]
                        .rearrange("r n c -> (r n) c") for b in range(NB)]
                T1Fb = [T1F[b * rpb:(b + 1) * rpb, :, :]
                        .rearrange("r n c -> (r n) c") for b in range(NB)]

                # -------- L1 edge phase + fused T1 build
                with tc.tile_pool(name="e_g", bufs=3) as gp, \
                     tc.tile_pool(name="e_ix", bufs=3) as ixp, \
                     tc.tile_pool(name="e_oh", bufs=4) as ohp, \
                     tc.tile_pool(name="e_ps", bufs=2, space="PSUM") as psp, \
                     tc.tile_pool(name="e_tp", bufs=2, space="PSUM") as tpp, \
                     tc.tile_pool(name="e_bp", bufs=2, space="PSUM") as bpp, \
                     tc.tile_pool(name="e_sb", bufs=3) as esb:
                    # zero the gather buffers once: pad slots are skipped by
                    # the gather (idx=-1) and must never hold non-finite bits
                    # (0 * NaN = NaN in the aggregation matmul).
                    for zi in range(3):
                        z = gp.tile([P, TPTmax, C0], DT0, tag="g",
                                    name=f"zg1_{zi}")
                        nc.vector.memset(z[:], 0.0)
                    for nt in range(NT):
                        tpt = TPT[nt]
                        c0 = CUM[nt]
                        wm = tw(nt)
                        g = gp.tile([P, TPTmax, C0], DT0, tag="g")
                        ixf = ixp.tile([P, TPTmax * 8], I16, tag="ixf")
                        nc.sync.dma_start(
                            out=ixf[:, :tpt * 8],
                            in_=idxf_d[:, c0 * 8:(c0 + tpt) * 8])
                        for b in range(NB):
                            for z0 in range(0, MX[nt][b], CH):
                                sz = min(CH, MX[nt][b] - z0)
                                o8 = (OFF[nt][b] - CUM[nt] * P + z0) // 16
                                ot = (OFF[nt][b] - CUM[nt] * P + z0) // P
                                nc.gpsimd.dma_gather(
                                    out_ap=g[:, ot:ot + (sz + P - 1) // P, :],
                                    in_ap=T0Fb[b],
                                    idxs_ap=ixf[:, o8:o8 + (sz + 15) // 16],
                                    num_idxs=sz, num_idxs_reg=sz,
                                    elem_size=C0, elem_step=C0,
                                    single_packet=False,
                                    queue_num=qrr[0] % nqueues)
                                qrr[0] += 1
                        # aggregate with w-weighted one-hots over the
                        # active (non-all-pad) columns only
                        acts = [t for b in range(NB)
                                for t in range((OFF[nt][b] - CUM[nt] * P) // P,
                                               (OFF[nt][b] - CUM[nt] * P) // P
                                               + (MX[nt][b] + P - 1) // P)]
                        if not acts:
                            acts = [0]
                        ps = psp.tile([P, D0], F32, tag="ps", space="PSUM")
                        for i, t in enumerate(acts):
                            oh = ohp.tile([P, P], DT0, tag="oh")
                            nc.vector.tensor_scalar(
                                out=oh[:], in0=iota_t[:],
                                scalar1=srel_t[:, c0 + t:c0 + t + 1],
                                scalar2=w0s_t[:, c0 + t:c0 + t + 1],
                                op0=OP.is_equal, op1=OP.mult)
                            nc.tensor.matmul(out=ps[:], lhsT=oh[:],
                                             rhs=g[:, t, :],
                                             start=(i == 0),
                                             stop=(i == len(acts) - 1))
                        hsb = esb.tile([P, D0], F32, tag="hsb")
                        nc.scalar.activation(hsb[:], ps[:], AF.Copy,
                                             scale=rec0_t[:, nt:nt + 1])
                        # fused: T1 row = [H1 @ W1 | f2' | 1 | pad], T1S = f1'
                        bp = bpp.tile([P, D1 + 2], F32, tag="bp", space="PSUM")
                        for k in range(KD0):
                            tp = tpp.tile([P, P], F32, tag="tp", space="PSUM")
                            nc.tensor.transpose(out=tp[:, :wm],
                                                in_=hsb[:wm, k * P:(k + 1) * P],
                                                identity=ident[:wm, :wm])
                            ht = esb.tile([P, P], F32, tag="ht")
                            nc.scalar.copy(out=ht[:, :wm], in_=tp[:, :wm])
                            nc.tensor.matmul(out=bp[:wm, :], lhsT=ht[:, :wm],
                                             rhs=w1e_t[k][:],
                                             start=(k == 0), stop=(k == KD0 - 1))
                        st = esb.tile([P, C1], DT1, tag="st2")
                        nc.scalar.copy(out=st[:wm, 0:D1 + 1],
                                       in_=bp[:wm, 0:D1 + 1])
                        nc.vector.memset(st[:, D1 + 1:D1 + 2], 1.0)
                        nc.vector.memset(st[:, D1 + 2:C1], 0.0)
                        nc.sync.dma_start(out=T1L[nt * P:nt * P + wm, :],
                                          in_=st[:wm, :])
                        sc = esb.tile([P, CS], BF16, tag="sc2")
                        nc.vector.memset(sc[:, :], 0.0)
                        nc.scalar.copy(out=sc[:wm, 0:1],
                                       in_=bp[:wm, D1 + 1:D1 + 2])
                        nc.sync.dma_start(out=T1S[nt * P:nt * P + wm, :],
                                          in_=sc[:wm, :])
                        if agsplit and nt == half_tile:
                            ag(T1L[0:HALF, :], T1F[:, 0:HALF, :], "t1a")
                    if agsplit:
                        ag(T1L[HALF:NSH, :], T1F[:, HALF:NSH, :], "t1b")
                    else:
                        ag(T1L[:, :], T1F[:, :, :], "t1")

                # -------- L2 edge phase + fused projection
                with tc.tile_pool(name="f_g", bufs=3) as gp2, \
                     tc.tile_pool(name="p_gs", bufs=2) as gsp, \
                     tc.tile_pool(name="p_ix", bufs=2) as ixps, \
                     tc.tile_pool(name="f_ix", bufs=3) as ixp2, \
                     tc.tile_pool(name="f_w", bufs=2) as wp, \
                     tc.tile_pool(name="f_oh", bufs=4) as ohp2, \
                     tc.tile_pool(name="f_ps", bufs=2, space="PSUM") as psp2, \
                     tc.tile_pool(name="f_tp", bufs=2, space="PSUM") as tpp2, \
                     tc.tile_pool(name="f_fp", bufs=2, space="PSUM") as cfp, \
                     tc.tile_pool(name="f_sb", bufs=3) as esb2:

                    def gs_pre(nt):
                        # f1'[src] from the core-local scalar table into
                        # f1all. Depends only on T1S (ready mid-L1), so the
                        # leading K0 tiles overlap the T1 all-gather.
                        tpt = TPT[nt]
                        c0 = CUM[nt]
                        gs = gsp.tile([P, TPTmax, CS], BF16, tag="gs",
                                      name="gs")
                        ixs = ixps.tile([P, TPTmax * 8], I16, tag="ixs",
                                        name="ixs")
                        nc.sync.dma_start(
                            out=ixs[:, :tpt * 8],
                            in_=idxs_d[:, c0 * 8:(c0 + tpt) * 8])
                        for q0 in range(0, tpt, 32):
                            qn = min(32, tpt - q0)
                            nc.gpsimd.dma_gather(
                                out_ap=gs[:, q0:q0 + qn, :], in_ap=T1S[:, :],
                                idxs_ap=ixs[:, q0 * 8:(q0 + qn) * 8],
                                num_idxs=qn * P, num_idxs_reg=qn * P,
                                elem_size=CS, single_packet=False,
                                queue_num=qrr[0] % nqueues)
                            qrr[0] += 1
                        nc.scalar.copy(out=f1all[:, c0:c0 + tpt],
                                       in_=gs[:, :tpt, 0])

                    # one-time zero of the gather pools so pad slots can never
                    # hold non-finite garbage (reused buffers stay finite).
                    for zi in range(3):
                        z = gp2.tile([P, TPTmax, C1], DT1, tag="g2",
                                     name=f"zg2_{zi}")
                        nc.vector.memset(z[:], 0.0)
                    for zi in range(2):
                        z2 = gsp.tile([P, TPTmax, CS], BF16, tag="gs",
                                      name=f"zgs_{zi}")
                        nc.vector.memset(z2[:], 0.0)
                    K0 = min(NT, int(os.environ.get("KPRE", "56")))
                    for nt in range(K0):
                        gs_pre(nt)
                    for nt in range(NT):
                        if K0 + nt < NT:
                            gs_pre(K0 + nt)
                        tpt = TPT[nt]
                        c0 = CUM[nt]
                        wm = tw(nt)
                        g = gp2.tile([P, TPTmax, C1], DT1, tag="g2")
                        ixf = ixp2.tile([P, TPTmax * 8], I16, tag="ixf2")
                        nc.sync.dma_start(
                            out=ixf[:, :tpt * 8],
                            in_=idxf_d[:, c0 * 8:(c0 + tpt) * 8])
                        for b in range(NB):
                            for z0 in range(0, MX[nt][b], CH):
                                sz = min(CH, MX[nt][b] - z0)
                                o8 = (OFF[nt][b] - CUM[nt] * P + z0) // 16
                                ot = (OFF[nt][b] - CUM[nt] * P + z0) // P
                                nc.gpsimd.dma_gather(
                                    out_ap=g[:, ot:ot + (sz + P - 1) // P, :],
                                    in_ap=T1Fb[b],
                                    idxs_ap=ixf[:, o8:o8 + (sz + 15) // 16],
                                    num_idxs=sz, num_idxs_reg=sz,
                                    elem_size=C1, elem_step=C1,
                                    single_packet=False,
                                    queue_num=qrr[0] % nqueues)
                                qrr[0] += 1
                        # w = exp(sigmoid(aval*(f1+f2))) = exp(.5*tanh(.5x)+.5)
                        # one-hot matmuls over active (non-all-pad) columns
                        acts = [t for b in range(NB)
                                for t in range((OFF[nt][b] - CUM[nt] * P) // P,
                                               (OFF[nt][b] - CUM[nt] * P) // P
                                               + (MX[nt][b] + P - 1) // P)]
                        if not acts:
                            acts = [0]
                        w = wp.tile([P, TPTmax], F32, tag="w")
                        nc.vector.tensor_tensor(out=w[:, :tpt],
                                                in0=f1all[:, c0:c0 + tpt],
                                                in1=g[:, :tpt, D1], op=OP.add)
                        nc.vector.tensor_tensor(out=w[:, :tpt], in0=w[:, :tpt],
                                                in1=aval_t[:, c0:c0 + tpt],
                                                op=OP.mult)
                        nc.scalar.activation(w[:, :tpt], w[:, :tpt], AF.Tanh,
                                             scale=0.5)
                        nc.scalar.activation(w[:, :tpt], w[:, :tpt], AF.Exp,
                                             scale=0.5, bias=half_col[:, :1])
                        ps = psp2.tile([P, D1 + 2], F32, tag="ps2", space="PSUM")
                        for i, t in enumerate(acts):
                            oh = ohp2.tile([P, P], DT1, tag="oh2")
                            nc.vector.tensor_scalar(
                                out=oh[:], in0=iota_t[:],
                                scalar1=srel_t[:, c0 + t:c0 + t + 1],
                                scalar2=w[:, t:t + 1],
                                op0=OP.is_equal, op1=OP.mult)
                            nc.tensor.matmul(out=ps[:], lhsT=oh[:],
                                             rhs=g[:, t, 0:D1 + 2],
                                             start=(i == 0),
                                             stop=(i == len(acts) - 1))
                        den = esb2.tile([P, 1], F32, tag="den")
                        nc.vector.tensor_scalar(out=den[:], in0=ps[:, D1 + 1:D1 + 2],
                                                scalar1=1e-30, scalar2=None,
                                                op0=OP.add)
                        rec = esb2.tile([P, 1], F32, tag="rec")
                        nc.vector.reciprocal(rec[:], den[:])
                        hsb = esb2.tile([P, D1], F32, tag="hsb2")
                        nc.scalar.activation(hsb[:], ps[:, 0:D1], AF.Copy,
                                             scale=rec[:, :1])
                        # fused projection: out = [H2 | nfm] @ proj + b
                        n0 = nt * P
                        tp = tpp2.tile([P, P], F32, tag="tp2", space="PSUM")
                        nc.tensor.transpose(out=tp[:, :wm], in_=hsb[:wm, 0:D1],
                                            identity=ident[:wm, :wm])
                        h2t = esb2.tile([P, P], F32, tag="h2t")
                        nc.scalar.copy(out=h2t[:, :wm], in_=tp[:, :wm])
                        nft = esb2.tile([FM, P], F32, tag="nft")
                        nc.sync.dma_start(out=nft[:, :wm], in_=NFMT[:, n0:n0 + wm])
                        fps = cfp.tile([P, NCLS], F32, tag="fps", space="PSUM")
                        nc.tensor.matmul(out=fps[:wm, :], lhsT=h2t[:, :wm],
                                         rhs=pja_t[:], start=True, stop=False)
                        nc.tensor.matmul(out=fps[:wm, :],
                                         lhsT=nft[:, :wm],
                                         rhs=pjb_t[:], start=False, stop=False)
                        nc.tensor.matmul(out=fps[:wm, :], lhsT=ones_row[:1, :wm],
                                         rhs=pbias_t[:], start=False, stop=True)
                        ot2 = esb2.tile([P, NCLS], F32, tag="ot")
                        nc.scalar.copy(out=ot2[:wm, :], in_=fps[:wm, :])
                        nc.sync.dma_start(out=out_d[n0:n0 + wm, :], in_=ot2[:wm, :])

            for _rep in range(reps):
                _body()

    nc.finalize()
    return nc


_CACHE = {}


def _get_program(cfg_key, cfg):
    if cfg_key not in _CACHE:
        _CACHE[cfg_key] = _build(cfg)
    return _CACHE[cfg_key]


def kernel(**inputs) -> np.ndarray:
    cfg, in_maps = _prep(inputs)
    cfg_key = (cfg["N"], cfg["E"], cfg["Din"], cfg["D0"], cfg["D1"],
               cfg["FM"], cfg["NCLS"], tuple(cfg["TPT"]),
               tuple(tuple(r) for r in cfg["SZ"]))
    nc = _get_program(cfg_key, cfg)
    res = run_bass_kernel_spmd(nc, in_maps, list(range(cfg["n_cores"])))
    out = np.concatenate(
        [res.results[c]["out"] for c in range(cfg["n_cores"])], axis=0)
    return out.astype(np.float32)
